# revision 7
# baseline (speedup 1.0000x reference)
"""Swin-style windowed-attention block (LN->W-MSA->residual->LN->MLP->residual)
for TRN2, data-parallel over batch across 8 NeuronCores.

Layout strategy: channels-on-partitions (CT) end to end; x arrives [B,C,H,W]
which is already channel-major per image. LayerNorm stats via ones-matmuls on
the PE (partition-dim sums), per-token scale/shift via DRAM-round-trip
partition broadcasts. Attention computes S-transposed (keys on partitions)
from per-head partition-0 tiles produced by an SBUF->SBUF DMA rearrangement;
softmax without max-subtraction (scores are small by construction); the
relative-position bias and the block-diagonal window mask are folded into one
host-precomputed multiplicative exp(bias) mask; row-sums ride along the AV
matmul as a ones-column of the value matrix; AV uses P~ as the stationary
operand giving token-major outputs that a cheap PE transpose returns to CT.
"""
import sys
import numpy as np
import ml_dtypes

sys.path.insert(0, "/opt/trn_rl_repo")

import concourse.bass as bass
import concourse.bacc as bacc
import concourse.tile as tile
from concourse import mybir
from concourse.bass_utils import run_bass_kernel_spmd

F32 = mybir.dt.float32
F32R = mybir.dt.float32r
BF16 = mybir.dt.bfloat16
AF = mybir.ActivationFunctionType
ALU = mybir.AluOpType
BF = ml_dtypes.bfloat16

# problem constants (hardcoded per the task contract)
B, C, H, W = 32, 512, 32, 32
NH, WS = 16, 4
HD = C // NH            # 32
N = WS * WS             # 16 tokens per window
EPS = 1e-5
MLP_H = 4 * C           # 2048
NCORES = 8
BI = B // NCORES        # images per core = 4
T = H * W               # tokens per image = 1024
NG = T // 128           # 128-token groups per image = 8

_cache = {}


def _relative_position_index(ws):
    coords = np.stack(np.meshgrid(np.arange(ws), np.arange(ws), indexing="ij"))
    cf = coords.reshape(2, -1)
    rel = cf[:, :, None] - cf[:, None, :]
    rel = rel.transpose(1, 2, 0).astype(np.int64)
    rel[:, :, 0] += ws - 1
    rel[:, :, 1] += ws - 1
    rel[:, :, 0] *= 2 * ws - 1
    return rel.sum(-1)


def _ap(t, off, dims):
    return bass.AP(tensor=t.tensor, offset=t.offset + off,
                   ap=[t.ap[0]] + [list(d) for d in dims])

# window permutation: window-ordered col = 128g + 16ww + 4i + j
#                     raster col         = 128g + 32i + 4ww + j
WIN4 = [[128, 4], [16, 8], [4, 4], [1, 4]]    # half-image (4 groups)
RAS4 = [[128, 4], [4, 8], [32, 4], [1, 4]]
WIN8 = [[128, 8], [16, 8], [4, 4], [1, 4]]    # full image (8 groups)
RAS8 = [[128, 8], [4, 8], [32, 4], [1, 4]]


def _build_program():
    nc = bacc.Bacc("TRN2", target_bir_lowering=False, debug=False,
                   enable_asserts=True, num_devices=NCORES)

    def din(name, shape, dt):
        return nc.dram_tensor(name, shape, dt, kind="ExternalInput").ap()

    x_d = din("x", (BI, C, H, W), F32)
    wqkv_d = din("wqkv", (C, 3 * C), BF16)        # [c, f] = diag(g1) @ Wqkv.T
    dq_d = din("dq", (128, 8), F32)               # q/k bias, col per f-tile
    dv_d = din("dvrow", (1, C), F32)              # v bias row (bcast source)
    wproj_d = din("wproj", (C, C), BF16)
    pb_d = din("pb", (128, 4), F32)
    wm1_d = din("wm1", (C, MLP_H), BF16)
    d1_d = din("d1", (128, 16), F32)
    wm2_d = din("wm2", (MLP_H, C), BF16)
    b2_d = din("b2", (128, 4), F32)
    mask_d = din("mask", (128, NH * 128), BF16)
    idm_d = din("idm", (128, 128), BF16)
    ones_d = din("onesc", (128, 1), F32)

    out_d = nc.dram_tensor("out", (BI, C, H, W), F32, kind="ExternalOutput").ap()

    with tile.TileContext(nc) as tc:
        with tc.tile_pool(name="sb", bufs=1) as sb, \
             tc.tile_pool(name="dr", bufs=1, space="DRAM") as dr, \
             tc.tile_pool(name="ps", bufs=1, space="PSUM") as ps:

            # ---------------- resident constants ----------------
            wqkv = []
            for c4 in range(4):
                wq_t = sb.tile([128, 3 * C], BF16, tag="wqkv", bufs=4,
                               name=f"wqkv{c4}")
                nc.sync.dma_start(out=wq_t, in_=wqkv_d[128 * c4:128 * (c4 + 1), :])
                wqkv.append(wq_t)
            wproj = []
            for c4 in range(4):
                wp_t = sb.tile([128, C], BF16, tag="wproj", bufs=4,
                               name=f"wproj{c4}")
                nc.sync.dma_start(out=wp_t, in_=wproj_d[128 * c4:128 * (c4 + 1), :])
                wproj.append(wp_t)
            wm1 = []
            for c4 in range(4):
                w1_t = sb.tile([128, MLP_H], BF16, tag="wm1", bufs=4,
                               name=f"wm1{c4}")
                nc.sync.dma_start(out=w1_t, in_=wm1_d[128 * c4:128 * (c4 + 1), :])
                wm1.append(w1_t)
            wm2 = []
            for c16 in range(16):
                w2_t = sb.tile([128, C], BF16, tag="wm2", bufs=16,
                               name=f"wm2{c16}")
                nc.sync.dma_start(out=w2_t, in_=wm2_d[128 * c16:128 * (c16 + 1), :])
                wm2.append(w2_t)
            mask_t = sb.tile([128, NH * 128], BF16, tag="mask", name="mask_t")
            nc.sync.dma_start(out=mask_t, in_=mask_d)
            idm = sb.tile([128, 128], BF16, tag="idm", name="idm")
            nc.sync.dma_start(out=idm, in_=idm_d)
            onesr = sb.tile([128, 1], F32R, tag="onesr", name="onesr")
            nc.sync.dma_start(out=onesr, in_=ones_d.bitcast(F32R))
            onesb = sb.tile([128, 1], BF16, tag="onesb", name="onesb")
            nc.vector.memset(onesb, 1.0)
            dq_t = sb.tile([128, 8], F32, tag="dq", name="dq_t")
            nc.sync.dma_start(out=dq_t, in_=dq_d)
            pb_t = sb.tile([128, 4], F32, tag="pbt", name="pb_t")
            nc.sync.dma_start(out=pb_t, in_=pb_d)
            d1_t = sb.tile([128, 16], F32, tag="d1t", name="d1_t")
            nc.sync.dma_start(out=d1_t, in_=d1_d)
            b2_t = sb.tile([128, 4], F32, tag="b2t", name="b2_t")
            nc.sync.dma_start(out=b2_t, in_=b2_d)
            dvb = sb.tile([128, C], F32, tag="dvb", name="dvb")
            nc.sync.dma_start(out=dvb, in_=bass.AP(
                tensor=dv_d.tensor, offset=dv_d.offset, ap=[[0, 128], [1, C]]))
            eps_t = sb.tile([1, 1], F32, tag="eps", name="eps_t")
            nc.vector.memset(eps_t, EPS)

            def layernorm(xc, sfx):
                """xc: 4 chunk tiles [128,1024] F32R -> (r_bc, mur_bc)."""
                sx = ps.tile([128, 1024], F32, tag="big", bufs=2,
                             name=f"sx_{sfx}")
                for hh in range(2):
                    for c4 in range(4):
                        nc.tensor.matmul(
                            sx[0:1, 512 * hh:512 * (hh + 1)], onesr,
                            xc[c4][:, 512 * hh:512 * (hh + 1)],
                            start=(c4 == 0), stop=(c4 == 3))
                st = sb.tile([128, 1024], F32, tag="stat", bufs=2,
                             name=f"st_{sfx}")
                nc.scalar.activation(st[0:1, :], sx[0:1, :], AF.Copy,
                                     scale=1.0 / C)
                sx2 = ps.tile([128, 1024], F32, tag="big", bufs=2,
                              name=f"sx2_{sfx}")
                for c4 in range(4):
                    x2 = sb.tile([128, 1024], BF16, tag="x2", bufs=2,
                                 name=f"x2_{sfx}_{c4}")
                    nc.vector.tensor_mul(x2, xc[c4].bitcast(F32),
                                         xc[c4].bitcast(F32))
                    for hh in range(2):
                        nc.tensor.matmul(
                            sx2[0:1, 512 * hh:512 * (hh + 1)], onesb,
                            x2[:, 512 * hh:512 * (hh + 1)],
                            start=(c4 == 0), stop=(c4 == 3))
                nc.vector.tensor_mul(st[32:33, :], st[0:1, :], st[0:1, :])
                nc.vector.scalar_tensor_tensor(st[64:65, :], sx2[0:1, :],
                                               1.0 / C, st[32:33, :],
                                               ALU.mult, ALU.subtract)
                nc.scalar.activation(st[96:97, :], st[64:65, :], AF.Ln,
                                     bias=eps_t)
                nc.scalar.activation(st[32:33, :], st[96:97, :], AF.Exp,
                                     scale=-0.5)
                # mur = (sum(x)/C) * r ; psum in0 is exempt from the
                # same-base-partition constraint on two-SBUF-input ops
                nc.vector.scalar_tensor_tensor(st[64:65, :], sx[0:1, :],
                                               1.0 / C, st[32:33, :],
                                               ALU.mult, ALU.mult)
                row = dr.tile([2, 1024], F32, tag="rt", bufs=2,
                              name=f"row_{sfx}")
                nc.sync.dma_start(out=row[0:1, :], in_=st[32:33, :])
                nc.sync.dma_start(out=row[1:2, :], in_=st[64:65, :])
                r_bc = sb.tile([128, 1024], F32, tag="bc", bufs=2,
                               name=f"rbc_{sfx}")
                nc.sync.dma_start(out=r_bc, in_=bass.AP(
                    tensor=row.tensor, offset=row.offset,
                    ap=[[0, 128], [1, 1024]]))
                mur_bc = sb.tile([128, 1024], F32, tag="bc", bufs=2,
                                 name=f"mbc_{sfx}")
                nc.sync.dma_start(out=mur_bc, in_=bass.AP(
                    tensor=row.tensor, offset=row.offset + 1024,
                    ap=[[0, 128], [1, 1024]]))
                return r_bc, mur_bc

            def z_pass(xc, r_bc, mur_bc, sfx):
                zc = []
                for c4 in range(4):
                    t1 = sb.tile([128, 1024], BF16, tag="zt", bufs=1,
                                 name=f"t1_{sfx}_{c4}")
                    nc.vector.tensor_mul(t1, xc[c4].bitcast(F32), r_bc)
                    z = sb.tile([128, 1024], BF16, tag="z", bufs=6,
                                name=f"z_{sfx}_{c4}")
                    nc.vector.tensor_tensor(out=z, in0=t1, in1=mur_bc,
                                            op=ALU.subtract)
                    zc.append(z)
                return zc

            # ---------------- per-image pipeline ----------------
            for img in range(BI):
                xc = []
                for c4 in range(4):
                    xraw = sb.tile([128, 1024], F32, tag="xraw", bufs=2,
                                   name=f"xr_{img}_{c4}")
                    nc.sync.dma_start(
                        out=xraw,
                        in_=x_d[img, 128 * c4:128 * (c4 + 1), :, :]
                        .rearrange("c h w -> c (h w)"))
                    xt = sb.tile([128, 1024], F32R, tag="xc", bufs=5,
                                 name=f"x_{img}_{c4}")
                    nc.vector.tensor_copy(_ap(xt, 0, WIN8),
                                          _ap(xraw, 0, RAS8).bitcast(F32R))
                    xc.append(xt)

                r_bc, mur_bc = layernorm(xc, f"l1_{img}")
                zc = z_pass(xc, r_bc, mur_bc, f"l1_{img}")

                # ---- qkv q/k f-tiles, ordered so quarter qt completes early
                qk = {}
                for fi in (0, 4, 1, 5, 2, 6, 3, 7):
                    qkt = sb.tile([128, 1024], BF16, tag="qk", bufs=8,
                                  name=f"qk_{img}_{fi}")
                    for th in range(2):
                        mm = ps.tile([128, 512], F32, tag="mm", bufs=2,
                                     name=f"qkp_{img}_{fi}_{th}")
                        for c4 in range(4):
                            nc.tensor.matmul(
                                mm, wqkv[c4][:, 128 * fi:128 * (fi + 1)],
                                zc[c4][:, 512 * th:512 * (th + 1)],
                                start=(c4 == 0), stop=(c4 == 3))
                        nc.vector.tensor_scalar_add(
                            qkt[:, 512 * th:512 * (th + 1)], mm,
                            dq_t[:, fi:fi + 1])
                    qk[fi] = qkt

                # ---- v (token-major with interleaved ones column)
                vaug = []
                for g in range(NG):
                    mm = ps.tile([128, 512], F32, tag="mm", bufs=2,
                                 name=f"vp_{img}_{g}")
                    for c4 in range(4):
                        nc.tensor.matmul(
                            mm, zc[c4][:, 128 * g:128 * (g + 1)],
                            wqkv[c4][:, 2 * C:3 * C],
                            start=(c4 == 0), stop=(c4 == 3))
                    va = sb.tile([128, 33 * NH], BF16, tag="vaug", bufs=8,
                                 name=f"va_{img}_{g}")
                    nc.vector.memset(_ap(va, 32, [[33, NH]]), 1.0)
                    nc.vector.tensor_tensor(
                        out=_ap(va, 0, [[33, NH], [1, 32]]),
                        in0=_ap(mm, 0, [[32, NH], [1, 32]]),
                        in1=_ap(dvb, 0, [[32, NH], [1, 32]]),
                        op=ALU.add)
                    vaug.append(va)

                # ---- attention, per head-quarter
                atc = [sb.tile([128, 512], BF16, tag="atc", bufs=9,
                               name=f"atc_{img}_{g}") for g in range(NG)]
                for qt in range(4):
                    qh = sb.tile([32, 4 * 1024], BF16, tag="qh", bufs=1,
                                 name=f"qh_{img}_{qt}")
                    kh = sb.tile([32, 4 * 1024], BF16, tag="kh", bufs=1,
                                 name=f"kh_{img}_{qt}")
                    for b4 in range(4):
                        nc.sync.dma_start(
                            out=qh[0:32, 1024 * b4:1024 * (b4 + 1)],
                            in_=qk[qt][32 * b4:32 * (b4 + 1), :])
                        nc.sync.dma_start(
                            out=kh[0:32, 1024 * b4:1024 * (b4 + 1)],
                            in_=qk[4 + qt][32 * b4:32 * (b4 + 1), :])
                    for g in range(NG):
                        stp = ps.tile([128, 512], F32, tag="mm", bufs=2,
                                      name=f"stp_{img}_{qt}_{g}")
                        for b4 in range(4):
                            sl = slice(1024 * b4 + 128 * g,
                                       1024 * b4 + 128 * (g + 1))
                            nc.tensor.matmul(
                                stp[:, 128 * b4:128 * (b4 + 1)],
                                kh[0:32, sl], qh[0:32, sl],
                                start=True, stop=True)
                        pt = sb.tile([128, 512], BF16, tag="pt", bufs=2,
                                     name=f"pt_{img}_{qt}_{g}")
                        nc.scalar.activation(pt, stp, AF.Exp)
                        nc.vector.tensor_mul(
                            pt, pt, mask_t[:, 512 * qt:512 * (qt + 1)])
                        av = ps.tile([128, 132], F32, tag="av", bufs=2,
                                     name=f"av_{img}_{qt}_{g}")
                        for b4 in range(4):
                            h = 4 * qt + b4
                            nc.tensor.matmul(
                                av[:, 33 * b4:33 * (b4 + 1)],
                                pt[:, 128 * b4:128 * (b4 + 1)],
                                vaug[g][:, 33 * h:33 * (h + 1)],
                                start=True, stop=True)
                        rec = sb.tile([128, 4], F32, tag="rec", bufs=4,
                                      name=f"rec_{img}_{qt}_{g}")
                        nc.vector.reciprocal(rec, _ap(av, 32, [[33, 4]]))
                        nc.vector.tensor_tensor(
                            out=_ap(atc[g], 128 * qt, [[32, 4], [1, 32]]),
                            in0=_ap(av, 0, [[33, 4], [1, 32]]),
                            in1=_ap(rec, 0, [[1, 4], [0, 32]]),
                            op=ALU.mult)

                # ---- transpose attention output to channel-major
                actn = []
                for fp in range(4):
                    at = sb.tile([128, 1024], BF16, tag="actn", bufs=4,
                                 name=f"actn_{img}_{fp}")
                    for Q in range(2):
                        tp = ps.tile([128, 512], BF16, tag="av", bufs=2,
                                     name=f"tp_{img}_{fp}_{Q}")
                        for gq in range(4):
                            g = 4 * Q + gq
                            nc.tensor.transpose(
                                tp[:, 128 * gq:128 * (gq + 1)],
                                atc[g][:, 128 * fp:128 * (fp + 1)], idm)
                        nc.scalar.copy(at[:, 512 * Q:512 * (Q + 1)], tp)
                    actn.append(at)

                # ---- proj + residual (in-place xh into xc, window->raster)
                for fo in range(4):
                    for th in range(2):
                        mm = ps.tile([128, 512], F32, tag="mm", bufs=2,
                                     name=f"pj_{img}_{fo}_{th}")
                        for c4 in range(4):
                            nc.tensor.matmul(
                                mm, wproj[c4][:, 128 * fo:128 * (fo + 1)],
                                actn[c4][:, 512 * th:512 * (th + 1)],
                                start=(c4 == 0), stop=(c4 == 3))
                        xap = xc[fo][:, 512 * th:512 * (th + 1)]
                        nc.vector.scalar_tensor_tensor(
                            xap, mm, pb_t[:, fo:fo + 1], xap,
                            ALU.add, ALU.add)

                r2_bc, mur2_bc = layernorm(xc, f"l2_{img}")
                z2c = z_pass(xc, r2_bc, mur2_bc, f"l2_{img}")

                # ---- MLP
                for th in range(2):
                    gel = []
                    for f16 in range(16):
                        mm = ps.tile([128, 512], F32, tag="mm", bufs=2,
                                     name=f"m1_{img}_{th}_{f16}")
                        for c4 in range(4):
                            nc.tensor.matmul(
                                mm, wm1[c4][:, 128 * f16:128 * (f16 + 1)],
                                z2c[c4][:, 512 * th:512 * (th + 1)],
                                start=(c4 == 0), stop=(c4 == 3))
                        gt = sb.tile([128, 512], BF16, tag="gelu", bufs=16,
                                     name=f"g_{img}_{th}_{f16}")
                        nc.scalar.activation(gt, mm, AF.Gelu,
                                             bias=d1_t[:, f16:f16 + 1])
                        gel.append(gt)
                    for fo in range(4):
                        mm2 = ps.tile([128, 1024], F32, tag="big", bufs=2,
                                      name=f"m2_{img}_{th}_{fo}")
                        for c16 in range(16):
                            nc.tensor.matmul(
                                mm2[:, 0:512],
                                wm2[c16][:, 128 * fo:128 * (fo + 1)],
                                gel[c16], start=(c16 == 0), stop=(c16 == 15))
                        xap = xc[fo][:, 512 * th:512 * (th + 1)]
                        nc.vector.scalar_tensor_tensor(
                            xap, mm2[:, 0:512], b2_t[:, fo:fo + 1], xap,
                            ALU.add, ALU.add)

                # ---- store (permute window->raster, then contiguous DMA)
                for c4 in range(4):
                    xo = sb.tile([128, 1024], F32, tag="xraw", bufs=2,
                                 name=f"xo_{img}_{c4}")
                    nc.vector.tensor_copy(_ap(xo, 0, RAS8),
                                          _ap(xc[c4], 0, WIN8).bitcast(F32))
                    nc.sync.dma_start(
                        out=out_d[img, 128 * c4:128 * (c4 + 1), :, :]
                        .rearrange("c h w -> c (h w)"),
                        in_=xo)

    nc.compile()
    return nc


def _prep_weights(inputs):
    """Host-side weight preprocessing (numpy, ~ms)."""
    g1 = np.asarray(inputs["norm1_w"], np.float32)
    b1 = np.asarray(inputs["norm1_b"], np.float32)
    g2 = np.asarray(inputs["norm2_w"], np.float32)
    b2n = np.asarray(inputs["norm2_b"], np.float32)
    wqkv = np.array(inputs["qkv_w"], np.float32)              # [3C, C]
    bqkv = np.array(inputs["qkv_b"], np.float32)
    scale = HD ** -0.5
    wqkv[:C] *= scale
    bqkv = bqkv.copy()
    bqkv[:C] *= scale
    dqkv = wqkv @ b1 + bqkv                                   # [3C]
    wqkvT = (wqkv * g1[None, :]).T                            # [C, 3C]

    wproj = np.asarray(inputs["proj_w"], np.float32)          # [C, C]
    pb = np.asarray(inputs["proj_b"], np.float32)
    wm1 = np.asarray(inputs["mlp_w1"], np.float32)            # [MLP_H, C]
    d1 = wm1 @ b2n + np.asarray(inputs["mlp_b1"], np.float32)
    wm1T = (wm1 * g2[None, :]).T                              # [C, MLP_H]
    wm2 = np.asarray(inputs["mlp_w2"], np.float32)            # [C, MLP_H]
    b2o = np.asarray(inputs["mlp_b2"], np.float32)

    rpb = np.asarray(inputs["rpb_table"], np.float32)         # [(2ws-1)^2, NH]
    rel = _relative_position_index(WS)                        # [N, N] (n, m)
    bias = rpb[rel.reshape(-1)].reshape(N, N, NH)             # [n, m, h]
    eb = np.exp(bias)
    mask = np.zeros((128, NH, 128), np.float32)
    for wdx in range(8):
        # tile entry [k, h, q]: k = 16w + m, q = 16w + n -> eb[n, m, h]
        mask[16 * wdx:16 * (wdx + 1), :, 16 * wdx:16 * (wdx + 1)] = \
            eb.transpose(1, 2, 0)
    mask2d = np.ascontiguousarray(
        mask.reshape(128, NH * 128))

    return {
        "wqkv": np.ascontiguousarray(wqkvT).astype(BF),
        "dq": np.ascontiguousarray(
            dqkv[:2 * C].reshape(8, 128).T).astype(np.float32),
        "dvrow": dqkv[2 * C:].reshape(1, C).astype(np.float32),
        "wproj": np.ascontiguousarray(wproj.T).astype(BF),
        "pb": np.ascontiguousarray(pb.reshape(4, 128).T).astype(np.float32),
        "wm1": np.ascontiguousarray(wm1T).astype(BF),
        "d1": np.ascontiguousarray(d1.reshape(16, 128).T).astype(np.float32),
        "wm2": np.ascontiguousarray(wm2.T).astype(BF),
        "b2": np.ascontiguousarray(b2o.reshape(4, 128).T).astype(np.float32),
        "mask": mask2d.astype(BF),
        "idm": np.eye(128, dtype=BF),
        "onesc": np.ones((128, 1), np.float32),
    }


def get_program():
    if "nc" not in _cache:
        _cache["nc"] = _build_program()
    return _cache["nc"]


def make_in_maps(inputs):
    wmaps = _prep_weights(inputs)
    x_full = np.asarray(inputs["x"], np.float32)
    in_maps = []
    for core in range(NCORES):
        m = dict(wmaps)
        m["x"] = np.ascontiguousarray(x_full[BI * core:BI * (core + 1)])
        in_maps.append(m)
    return in_maps


def kernel(**inputs):
    nc = get_program()
    in_maps = make_in_maps(inputs)
    res = run_bass_kernel_spmd(nc, in_maps, list(range(NCORES)))
    out = np.concatenate([res.results[c]["out"] for c in range(NCORES)],
                         axis=0)
    return out


# revision 18
# speedup vs baseline: 4898.5529x; 4898.5529x over previous
"""Swin-style windowed-attention block (LN->W-MSA->residual->LN->MLP->residual)
for TRN2, data-parallel over batch across 8 NeuronCores.

Layout strategy: channels-on-partitions (CT) end to end; x arrives [B,C,H,W]
which is already channel-major per image. LayerNorm stats via ones-matmuls on
the PE (partition-dim sums), per-token scale/shift via DRAM-round-trip
partition broadcasts. Attention computes S-transposed (keys on partitions)
from per-head partition-0 tiles produced by an SBUF->SBUF DMA rearrangement;
softmax without max-subtraction (scores are small by construction); the
relative-position bias and the block-diagonal window mask are folded into one
host-precomputed multiplicative exp(bias) mask; row-sums ride along the AV
matmul as a ones-column of the value matrix; AV uses P~ as the stationary
operand giving token-major outputs that a cheap PE transpose returns to CT.
"""
import sys
import numpy as np
import ml_dtypes

sys.path.insert(0, "/opt/trn_rl_repo")

import concourse.bass as bass
import concourse.bacc as bacc
import concourse.tile as tile
from concourse import mybir
from concourse.bass_utils import run_bass_kernel_spmd

F32 = mybir.dt.float32
F32R = mybir.dt.float32r
BF16 = mybir.dt.bfloat16
AF = mybir.ActivationFunctionType
ALU = mybir.AluOpType
BF = ml_dtypes.bfloat16

# problem constants (hardcoded per the task contract)
B, C, H, W = 32, 512, 32, 32
NH, WS = 16, 4
HD = C // NH            # 32
N = WS * WS             # 16 tokens per window
EPS = 1e-5
MLP_H = 4 * C           # 2048
NCORES = 8
BI = B // NCORES        # images per core = 4
T = H * W               # tokens per image = 1024
NG = T // 128           # 128-token groups per image = 8

_cache = {}


def _relative_position_index(ws):
    coords = np.stack(np.meshgrid(np.arange(ws), np.arange(ws), indexing="ij"))
    cf = coords.reshape(2, -1)
    rel = cf[:, :, None] - cf[:, None, :]
    rel = rel.transpose(1, 2, 0).astype(np.int64)
    rel[:, :, 0] += ws - 1
    rel[:, :, 1] += ws - 1
    rel[:, :, 0] *= 2 * ws - 1
    return rel.sum(-1)


def _ap(t, off, dims):
    return bass.AP(tensor=t.tensor, offset=t.offset + off,
                   ap=[t.ap[0]] + [list(d) for d in dims])

# window permutation: window-ordered col = 128g + 16ww + 4i + j
#                     raster col         = 128g + 32i + 4ww + j
WIN4 = [[128, 4], [16, 8], [4, 4], [1, 4]]    # half-image (4 groups)
RAS4 = [[128, 4], [4, 8], [32, 4], [1, 4]]
WIN8 = [[128, 8], [16, 8], [4, 4], [1, 4]]    # full image (8 groups)
RAS8 = [[128, 8], [4, 8], [32, 4], [1, 4]]


def _build_program():
    nc = bacc.Bacc("TRN2", target_bir_lowering=False, debug=False,
                   enable_asserts=True, num_devices=NCORES)

    def din(name, shape, dt):
        return nc.dram_tensor(name, shape, dt, kind="ExternalInput").ap()

    x_d = din("x", (BI, C, H, W), F32)
    wqkv_d = din("wqkv", (C, 3 * C), BF16)        # [c, f] = diag(g1) @ Wqkv.T
    dq_d = din("dq", (128, 8), F32)               # q/k bias, col per f-tile
    dv_d = din("dvrow", (1, C), F32)              # v bias row (bcast source)
    wproj_d = din("wproj", (C, C), BF16)
    pb_d = din("pb", (128, 4), F32)
    wm1_d = din("wm1", (C, MLP_H), BF16)
    d1_d = din("d1", (128, 16), F32)
    wm2_d = din("wm2", (MLP_H, C), BF16)
    b2_d = din("b2", (128, 4), F32)
    mask_d = din("mask", (128, NH * 128), BF16)
    idm_d = din("idm", (128, 128), BF16)
    ones_d = din("onesc", (128, 1), F32)

    out_d = nc.dram_tensor("out", (BI, C, H, W), F32, kind="ExternalOutput").ap()

    with tile.TileContext(nc) as tc:
        with tc.tile_pool(name="sb", bufs=1) as sb, \
             tc.tile_pool(name="dr", bufs=1, space="DRAM") as dr, \
             tc.tile_pool(name="ps", bufs=1, space="PSUM") as ps:

            # ---------------- resident constants ----------------
            wqkv = []
            for c4 in range(4):
                wq_t = sb.tile([128, 3 * C], BF16, tag="wqkv", bufs=4,
                               name=f"wqkv{c4}")
                nc.sync.dma_start(out=wq_t, in_=wqkv_d[128 * c4:128 * (c4 + 1), :])
                wqkv.append(wq_t)
            wproj = []
            for c4 in range(4):
                wp_t = sb.tile([128, C], BF16, tag="wproj", bufs=4,
                               name=f"wproj{c4}")
                nc.sync.dma_start(out=wp_t, in_=wproj_d[128 * c4:128 * (c4 + 1), :])
                wproj.append(wp_t)
            wm1 = []
            for c4 in range(4):
                w1_t = sb.tile([128, MLP_H], BF16, tag="wm1", bufs=4,
                               name=f"wm1{c4}")
                nc.sync.dma_start(out=w1_t, in_=wm1_d[128 * c4:128 * (c4 + 1), :])
                wm1.append(w1_t)
            wm2 = []
            for c16 in range(16):
                w2_t = sb.tile([128, C], BF16, tag="wm2", bufs=16,
                               name=f"wm2{c16}")
                nc.sync.dma_start(out=w2_t, in_=wm2_d[128 * c16:128 * (c16 + 1), :])
                wm2.append(w2_t)
            mask_t = sb.tile([128, NH * 128], BF16, tag="mask", name="mask_t")
            nc.sync.dma_start(out=mask_t, in_=mask_d)
            idm = sb.tile([128, 128], BF16, tag="idm", name="idm")
            nc.sync.dma_start(out=idm, in_=idm_d)
            onesr = sb.tile([128, 1], F32R, tag="onesr", name="onesr")
            nc.sync.dma_start(out=onesr, in_=ones_d.bitcast(F32R))
            onesb = sb.tile([128, 1], BF16, tag="onesb", name="onesb")
            nc.vector.memset(onesb, 1.0)
            dq_t = sb.tile([128, 8], F32, tag="dq", name="dq_t")
            nc.sync.dma_start(out=dq_t, in_=dq_d)
            pb_t = sb.tile([128, 4], F32, tag="pbt", name="pb_t")
            nc.sync.dma_start(out=pb_t, in_=pb_d)
            d1_t = sb.tile([128, 16], F32, tag="d1t", name="d1_t")
            nc.sync.dma_start(out=d1_t, in_=d1_d)
            b2_t = sb.tile([128, 4], F32, tag="b2t", name="b2_t")
            nc.sync.dma_start(out=b2_t, in_=b2_d)
            dvb = sb.tile([128, C], F32, tag="dvb", name="dvb")
            nc.sync.dma_start(out=dvb, in_=bass.AP(
                tensor=dv_d.tensor, offset=dv_d.offset, ap=[[0, 128], [1, C]]))
            eps_t = sb.tile([1, 1], F32, tag="eps", name="eps_t")
            nc.vector.memset(eps_t, EPS)

            def layernorm(xc, sfx):
                """xc: 4 chunk tiles [128,1024] F32R -> (r_bc, mur_bc)."""
                sx = ps.tile([128, 1024], F32, tag="sx", bufs=1,
                             name=f"sx_{sfx}")
                for hh in range(2):
                    for c4 in range(4):
                        nc.tensor.matmul(
                            sx[0:1, 512 * hh:512 * (hh + 1)], onesr,
                            xc[c4][:, 512 * hh:512 * (hh + 1)],
                            start=(c4 == 0), stop=(c4 == 3))
                st = sb.tile([128, 1024], F32, tag="stat", bufs=2,
                             name=f"st_{sfx}")
                sx2 = ps.tile([128, 1024], F32, tag="sx2", bufs=1,
                              name=f"sx2_{sfx}")
                for c4 in range(4):
                    x2 = sb.tile([128, 1024], BF16, tag="zt", bufs=2,
                                 name=f"x2_{sfx}_{c4}")
                    nc.vector.tensor_mul(x2, xc[c4].bitcast(F32),
                                         xc[c4].bitcast(F32))
                    for hh in range(2):
                        nc.tensor.matmul(
                            sx2[0:1, 512 * hh:512 * (hh + 1)], onesb,
                            x2[:, 512 * hh:512 * (hh + 1)],
                            start=(c4 == 0), stop=(c4 == 3))
                # mu2 = (sum(x)/C)^2 straight from psum
                nc.scalar.activation(st[0:1, :], sx[0:1, :], AF.Square,
                                     scale=1.0 / C)
                nc.vector.scalar_tensor_tensor(st[64:65, :], sx2[0:1, :],
                                               1.0 / C, st[0:1, :],
                                               ALU.mult, ALU.subtract)
                nc.scalar.activation(st[96:97, :], st[64:65, :], AF.Ln,
                                     bias=eps_t)
                nc.scalar.activation(st[32:33, :], st[96:97, :], AF.Exp,
                                     scale=-0.5)
                # mur = (sum(x)/C) * r ; psum in0 is exempt from the
                # same-base-partition constraint on two-SBUF-input ops
                nc.vector.scalar_tensor_tensor(st[64:65, :], sx[0:1, :],
                                               1.0 / C, st[32:33, :],
                                               ALU.mult, ALU.mult)
                row = dr.tile([2, 1024], BF16, tag="rt", bufs=2,
                              name=f"row_{sfx}")
                # casting DMAs (gpsimd) write r and mur rows
                nc.gpsimd.dma_start(out=row[0:1, :], in_=st[32:33, :])
                nc.gpsimd.dma_start(out=row[1:2, :], in_=st[64:65, :])
                rm_bc = sb.tile([128, 2048], BF16, tag="bc", bufs=1,
                                name=f"rmbc_{sfx}")
                nc.sync.dma_start(out=rm_bc, in_=bass.AP(
                    tensor=row.tensor, offset=row.offset,
                    ap=[[0, 128], [1, 2048]]))
                return rm_bc[:, 0:1024], rm_bc[:, 1024:2048]

            def z_pass(xc, r_bc, mur_bc, sfx):
                zc = []
                for c4 in range(4):
                    t1 = sb.tile([128, 1024], BF16, tag="zt", bufs=2,
                                 name=f"t1_{sfx}_{c4}")
                    nc.vector.tensor_mul(t1, xc[c4].bitcast(F32), r_bc)
                    z = sb.tile([128, 1024], BF16, tag="z", bufs=8,
                                name=f"z_{sfx}_{c4}")
                    nc.vector.tensor_tensor(out=z, in0=t1, in1=mur_bc,
                                            op=ALU.subtract)
                    zc.append(z)
                return zc

            def load_ln1(img):
                xc = []
                for c4 in range(4):
                    xraw = sb.tile([128, 1024], F32, tag="xraw", bufs=2,
                                   name=f"xr_{img}_{c4}")
                    nc.sync.dma_start(
                        out=xraw,
                        in_=x_d[img, 128 * c4:128 * (c4 + 1), :, :]
                        .rearrange("c h w -> c (h w)"))
                    xt = sb.tile([128, 1024], F32R, tag="xc", bufs=8,
                                 name=f"x_{img}_{c4}")
                    nc.vector.tensor_copy(_ap(xt, 0, WIN8),
                                          _ap(xraw, 0, RAS8).bitcast(F32R))
                    xc.append(xt)
                r_bc, mur_bc = layernorm(xc, f"l1_{img}")
                zc = z_pass(xc, r_bc, mur_bc, f"l1_{img}")
                return xc, zc

            # ---------------- per-image pipeline ----------------
            nxt = load_ln1(0)
            for img in range(BI):
                xc, zc = nxt

                # ---- qkv q/k f-tiles
                qk = {}
                for fi in (0, 4, 1, 5, 2, 6, 3, 7):
                    qkt = sb.tile([128, 1024], BF16, tag="qk", bufs=7,
                                  name=f"qk_{img}_{fi}")
                    for th in range(2):
                        mm = ps.tile([128, 512], F32, tag="mm", bufs=2,
                                     name=f"qkp_{img}_{fi}_{th}")
                        for c4 in range(4):
                            nc.tensor.matmul(
                                mm, wqkv[c4][:, 128 * fi:128 * (fi + 1)],
                                zc[c4][:, 512 * th:512 * (th + 1)],
                                start=(c4 == 0), stop=(c4 == 3))
                        nc.vector.tensor_scalar_add(
                            qkt[:, 512 * th:512 * (th + 1)], mm,
                            dq_t[:, fi:fi + 1])
                    qk[fi] = qkt

                # ---- v (token-major with interleaved ones column)
                vaug = []
                for g in range(NG):
                    mm = ps.tile([128, 512], F32, tag="mm", bufs=2,
                                 name=f"vp_{img}_{g}")
                    for c4 in range(4):
                        nc.tensor.matmul(
                            mm, zc[c4][:, 128 * g:128 * (g + 1)],
                            wqkv[c4][:, 2 * C:3 * C],
                            start=(c4 == 0), stop=(c4 == 3))
                    va = sb.tile([128, 33 * NH], BF16, tag="vaug", bufs=8,
                                 name=f"va_{img}_{g}")
                    nc.vector.memset(_ap(va, 32, [[33, NH]]), 1.0)
                    nc.vector.tensor_tensor(
                        out=_ap(va, 0, [[33, NH], [1, 32]]),
                        in0=_ap(mm, 0, [[32, NH], [1, 32]]),
                        in1=_ap(dvb, 0, [[32, NH], [1, 32]]),
                        op=ALU.add)
                    vaug.append(va)

                # ---- attention, per head-quarter
                atc = [sb.tile([128, 512], BF16, tag="atc", bufs=8,
                               name=f"atc_{img}_{g}") for g in range(NG)]
                for qt in range(4):
                    qh = sb.tile([32, 4 * 1024], BF16, tag="qh", bufs=2,
                                 name=f"qh_{img}_{qt}")
                    kh = sb.tile([32, 4 * 1024], BF16, tag="kh", bufs=1,
                                 name=f"kh_{img}_{qt}")
                    for b4 in range(4):
                        nc.sync.dma_start(
                            out=qh[0:32, 1024 * b4:1024 * (b4 + 1)],
                            in_=qk[qt][32 * b4:32 * (b4 + 1), :])
                        nc.sync.dma_start(
                            out=kh[0:32, 1024 * b4:1024 * (b4 + 1)],
                            in_=qk[4 + qt][32 * b4:32 * (b4 + 1), :])
                    for g in range(NG):
                        stp = ps.tile([128, 512], F32, tag="mm", bufs=2,
                                      name=f"stp_{img}_{qt}_{g}")
                        for b4 in range(4):
                            sl = slice(1024 * b4 + 128 * g,
                                       1024 * b4 + 128 * (g + 1))
                            nc.tensor.matmul(
                                stp[:, 128 * b4:128 * (b4 + 1)],
                                kh[0:32, sl], qh[0:32, sl],
                                start=True, stop=True)
                        pt = sb.tile([128, 512], BF16, tag="pt", bufs=2,
                                     name=f"pt_{img}_{qt}_{g}")
                        nc.scalar.activation(pt, stp, AF.Exp)
                        nc.vector.tensor_mul(
                            pt, pt, mask_t[:, 512 * qt:512 * (qt + 1)])
                        av = ps.tile([128, 132], F32, tag="mm", bufs=2,
                                     name=f"av_{img}_{qt}_{g}")
                        for b4 in range(4):
                            h = 4 * qt + b4
                            nc.tensor.matmul(
                                av[:, 33 * b4:33 * (b4 + 1)],
                                pt[:, 128 * b4:128 * (b4 + 1)],
                                vaug[g][:, 33 * h:33 * (h + 1)],
                                start=True, stop=True)
                        rec = sb.tile([128, 4], F32, tag="rec", bufs=2,
                                      name=f"rec_{img}_{qt}_{g}")
                        nc.vector.reciprocal(rec, _ap(av, 32, [[33, 4]]))
                        nc.vector.tensor_tensor(
                            out=_ap(atc[g], 128 * qt, [[32, 4], [1, 32]]),
                            in0=_ap(av, 0, [[33, 4], [1, 32]]),
                            in1=_ap(rec, 0, [[1, 4], [0, 32]]),
                            op=ALU.mult)

                # ---- transpose attention output to channel-major
                actn = []
                for fp in range(4):
                    at = sb.tile([128, 1024], BF16, tag="actn", bufs=4,
                                 name=f"actn_{img}_{fp}")
                    for Q in range(2):
                        tp = ps.tile([128, 512], BF16, tag="av", bufs=1,
                                     name=f"tp_{img}_{fp}_{Q}")
                        for gq in range(4):
                            g = 4 * Q + gq
                            nc.tensor.transpose(
                                tp[:, 128 * gq:128 * (gq + 1)],
                                atc[g][:, 128 * fp:128 * (fp + 1)], idm)
                        nc.scalar.copy(at[:, 512 * Q:512 * (Q + 1)], tp)
                    actn.append(at)

                # ---- proj + residual (in-place xh into xc, window->raster)
                for fo in range(4):
                    for th in range(2):
                        mm = ps.tile([128, 512], F32, tag="mm", bufs=2,
                                     name=f"pj_{img}_{fo}_{th}")
                        for c4 in range(4):
                            nc.tensor.matmul(
                                mm, wproj[c4][:, 128 * fo:128 * (fo + 1)],
                                actn[c4][:, 512 * th:512 * (th + 1)],
                                start=(c4 == 0), stop=(c4 == 3))
                        xap = xc[fo][:, 512 * th:512 * (th + 1)]
                        nc.vector.scalar_tensor_tensor(
                            xap, mm, pb_t[:, fo:fo + 1], xap,
                            ALU.add, ALU.add)

                # prefetch next image's LN1 pipeline under this image's tail
                if img + 1 < BI:
                    nxt = load_ln1(img + 1)

                r2_bc, mur2_bc = layernorm(xc, f"l2_{img}")
                z2c = z_pass(xc, r2_bc, mur2_bc, f"l2_{img}")

                # ---- MLP
                for th in range(2):
                    gel = []
                    for f16 in range(16):
                        mm = ps.tile([128, 512], F32, tag="mm", bufs=2,
                                     name=f"m1_{img}_{th}_{f16}")
                        for c4 in range(4):
                            nc.tensor.matmul(
                                mm, wm1[c4][:, 128 * f16:128 * (f16 + 1)],
                                z2c[c4][:, 512 * th:512 * (th + 1)],
                                start=(c4 == 0), stop=(c4 == 3))
                        gt = sb.tile([128, 512], BF16, tag="gelu", bufs=16,
                                     name=f"g_{img}_{th}_{f16}")
                        nc.scalar.activation(gt, mm, AF.Gelu,
                                             bias=d1_t[:, f16:f16 + 1])
                        gel.append(gt)
                    for fo in range(4):
                        mm2 = ps.tile([128, 512], F32, tag="m2", bufs=1,
                                      name=f"m2_{img}_{th}_{fo}")
                        for c16 in range(16):
                            nc.tensor.matmul(
                                mm2, wm2[c16][:, 128 * fo:128 * (fo + 1)],
                                gel[c16], start=(c16 == 0), stop=(c16 == 15))
                        xap = xc[fo][:, 512 * th:512 * (th + 1)]
                        nc.vector.scalar_tensor_tensor(
                            xap, mm2, b2_t[:, fo:fo + 1], xap,
                            ALU.add, ALU.add)

                # ---- store (permute window->raster, then contiguous DMA)
                for c4 in range(4):
                    xo = sb.tile([128, 1024], F32, tag="xraw", bufs=2,
                                 name=f"xo_{img}_{c4}")
                    nc.vector.tensor_copy(_ap(xo, 0, RAS8),
                                          _ap(xc[c4], 0, WIN8).bitcast(F32))
                    nc.sync.dma_start(
                        out=out_d[img, 128 * c4:128 * (c4 + 1), :, :]
                        .rearrange("c h w -> c (h w)"),
                        in_=xo)

    nc.compile()
    return nc


def _prep_weights(inputs):
    """Host-side weight preprocessing (numpy, ~ms)."""
    g1 = np.asarray(inputs["norm1_w"], np.float32)
    b1 = np.asarray(inputs["norm1_b"], np.float32)
    g2 = np.asarray(inputs["norm2_w"], np.float32)
    b2n = np.asarray(inputs["norm2_b"], np.float32)
    wqkv = np.array(inputs["qkv_w"], np.float32)              # [3C, C]
    bqkv = np.array(inputs["qkv_b"], np.float32)
    scale = HD ** -0.5
    wqkv[:C] *= scale
    bqkv = bqkv.copy()
    bqkv[:C] *= scale
    dqkv = wqkv @ b1 + bqkv                                   # [3C]
    wqkvT = (wqkv * g1[None, :]).T                            # [C, 3C]

    wproj = np.asarray(inputs["proj_w"], np.float32)          # [C, C]
    pb = np.asarray(inputs["proj_b"], np.float32)
    wm1 = np.asarray(inputs["mlp_w1"], np.float32)            # [MLP_H, C]
    d1 = wm1 @ b2n + np.asarray(inputs["mlp_b1"], np.float32)
    wm1T = (wm1 * g2[None, :]).T                              # [C, MLP_H]
    wm2 = np.asarray(inputs["mlp_w2"], np.float32)            # [C, MLP_H]
    b2o = np.asarray(inputs["mlp_b2"], np.float32)

    rpb = np.asarray(inputs["rpb_table"], np.float32)         # [(2ws-1)^2, NH]
    rel = _relative_position_index(WS)                        # [N, N] (n, m)
    bias = rpb[rel.reshape(-1)].reshape(N, N, NH)             # [n, m, h]
    eb = np.exp(bias)
    mask = np.zeros((128, NH, 128), np.float32)
    for wdx in range(8):
        # tile entry [k, h, q]: k = 16w + m, q = 16w + n -> eb[n, m, h]
        mask[16 * wdx:16 * (wdx + 1), :, 16 * wdx:16 * (wdx + 1)] = \
            eb.transpose(1, 2, 0)
    mask2d = np.ascontiguousarray(
        mask.reshape(128, NH * 128))

    return {
        "wqkv": np.ascontiguousarray(wqkvT).astype(BF),
        "dq": np.ascontiguousarray(
            dqkv[:2 * C].reshape(8, 128).T).astype(np.float32),
        "dvrow": dqkv[2 * C:].reshape(1, C).astype(np.float32),
        "wproj": np.ascontiguousarray(wproj.T).astype(BF),
        "pb": np.ascontiguousarray(pb.reshape(4, 128).T).astype(np.float32),
        "wm1": np.ascontiguousarray(wm1T).astype(BF),
        "d1": np.ascontiguousarray(d1.reshape(16, 128).T).astype(np.float32),
        "wm2": np.ascontiguousarray(wm2.T).astype(BF),
        "b2": np.ascontiguousarray(b2o.reshape(4, 128).T).astype(np.float32),
        "mask": mask2d.astype(BF),
        "idm": np.eye(128, dtype=BF),
        "onesc": np.ones((128, 1), np.float32),
    }


def get_program():
    if "nc" not in _cache:
        _cache["nc"] = _build_program()
    return _cache["nc"]


def make_in_maps(inputs):
    wmaps = _prep_weights(inputs)
    x_full = np.asarray(inputs["x"], np.float32)
    in_maps = []
    for core in range(NCORES):
        m = dict(wmaps)
        m["x"] = np.ascontiguousarray(x_full[BI * core:BI * (core + 1)])
        in_maps.append(m)
    return in_maps


def kernel(**inputs):
    nc = get_program()
    in_maps = make_in_maps(inputs)
    res = run_bass_kernel_spmd(nc, in_maps, list(range(NCORES)))
    out = np.concatenate([res.results[c]["out"] for c in range(NCORES)],
                         axis=0)
    return out


# revision 22
# speedup vs baseline: 5584.7064x; 1.1401x over previous
"""Swin-style windowed-attention block (LN->W-MSA->residual->LN->MLP->residual)
for TRN2, data-parallel over batch across 8 NeuronCores.

Layout strategy: channels-on-partitions (CT) end to end; x arrives [B,C,H,W]
which is already channel-major per image. LayerNorm stats via ones-matmuls on
the PE (partition-dim sums), per-token scale/shift via DRAM-round-trip
partition broadcasts. Attention computes S-transposed (keys on partitions)
from per-head partition-0 tiles produced by an SBUF->SBUF DMA rearrangement;
softmax without max-subtraction (scores are small by construction); the
relative-position bias and the block-diagonal window mask are folded into one
host-precomputed multiplicative exp(bias) mask; row-sums ride along the AV
matmul as a ones-column of the value matrix; AV uses P~ as the stationary
operand giving token-major outputs that a cheap PE transpose returns to CT.
"""
import sys
import numpy as np
import ml_dtypes

sys.path.insert(0, "/opt/trn_rl_repo")

import concourse.bass as bass
import concourse.bacc as bacc
import concourse.tile as tile
from concourse import mybir
from concourse.bass_utils import run_bass_kernel_spmd

F32 = mybir.dt.float32
F32R = mybir.dt.float32r
BF16 = mybir.dt.bfloat16
AF = mybir.ActivationFunctionType
ALU = mybir.AluOpType
BF = ml_dtypes.bfloat16

# problem constants (hardcoded per the task contract)
B, C, H, W = 32, 512, 32, 32
NH, WS = 16, 4
HD = C // NH            # 32
N = WS * WS             # 16 tokens per window
EPS = 1e-5
MLP_H = 4 * C           # 2048
NCORES = 8
BI = B // NCORES        # images per core = 4
T = H * W               # tokens per image = 1024
NG = T // 128           # 128-token groups per image = 8

_cache = {}


def _relative_position_index(ws):
    coords = np.stack(np.meshgrid(np.arange(ws), np.arange(ws), indexing="ij"))
    cf = coords.reshape(2, -1)
    rel = cf[:, :, None] - cf[:, None, :]
    rel = rel.transpose(1, 2, 0).astype(np.int64)
    rel[:, :, 0] += ws - 1
    rel[:, :, 1] += ws - 1
    rel[:, :, 0] *= 2 * ws - 1
    return rel.sum(-1)


def _ap(t, off, dims):
    return bass.AP(tensor=t.tensor, offset=t.offset + off,
                   ap=[t.ap[0]] + [list(d) for d in dims])

# window permutation: window-ordered col = 128g + 16ww + 4i + j
#                     raster col         = 128g + 32i + 4ww + j
WIN4 = [[128, 4], [16, 8], [4, 4], [1, 4]]    # half-image (4 groups)
RAS4 = [[128, 4], [4, 8], [32, 4], [1, 4]]
WIN8 = [[128, 8], [16, 8], [4, 4], [1, 4]]    # full image (8 groups)
RAS8 = [[128, 8], [4, 8], [32, 4], [1, 4]]


def _build_program():
    nc = bacc.Bacc("TRN2", target_bir_lowering=False, debug=False,
                   enable_asserts=True, num_devices=NCORES)

    def din(name, shape, dt):
        return nc.dram_tensor(name, shape, dt, kind="ExternalInput").ap()

    x_d = din("x", (BI, C, H, W), F32)
    wqkv_d = din("wqkv", (C, 3 * C), BF16)        # [c, f] = diag(g1) @ Wqkv.T
    dq_d = din("dq", (128, 8), F32)               # q/k bias, col per f-tile
    dv_d = din("dvrow", (1, C), F32)              # v bias row (bcast source)
    wproj_d = din("wproj", (C, C), BF16)
    pb_d = din("pb", (128, 4), F32)
    wm1_d = din("wm1", (C, MLP_H), BF16)
    d1_d = din("d1", (128, 16), F32)
    wm2_d = din("wm2", (MLP_H, C), BF16)
    b2_d = din("b2", (128, 4), F32)
    mask_d = din("mask", (128, NH * 128), BF16)
    idm_d = din("idm", (128, 128), BF16)
    ones_d = din("onesc", (128, 1), F32)

    out_d = nc.dram_tensor("out", (BI, C, H, W), F32, kind="ExternalOutput").ap()

    with tile.TileContext(nc) as tc:
        with tc.tile_pool(name="sb", bufs=1) as sb, \
             tc.tile_pool(name="dr", bufs=1, space="DRAM") as dr, \
             tc.tile_pool(name="ps", bufs=1, space="PSUM") as ps:

            # ---------------- resident constants ----------------
            wqkv = []
            for c4 in range(4):
                wq_t = sb.tile([128, 3 * C], BF16, tag="wqkv", bufs=4,
                               name=f"wqkv{c4}")
                nc.sync.dma_start(out=wq_t, in_=wqkv_d[128 * c4:128 * (c4 + 1), :])
                wqkv.append(wq_t)
            wproj = []
            for c4 in range(4):
                wp_t = sb.tile([128, C], BF16, tag="wproj", bufs=4,
                               name=f"wproj{c4}")
                nc.sync.dma_start(out=wp_t, in_=wproj_d[128 * c4:128 * (c4 + 1), :])
                wproj.append(wp_t)
            wm1 = []
            for c4 in range(4):
                w1_t = sb.tile([128, MLP_H], BF16, tag="wm1", bufs=4,
                               name=f"wm1{c4}")
                nc.sync.dma_start(out=w1_t, in_=wm1_d[128 * c4:128 * (c4 + 1), :])
                wm1.append(w1_t)
            wm2 = []
            for c16 in range(16):
                w2_t = sb.tile([128, C], BF16, tag="wm2", bufs=16,
                               name=f"wm2{c16}")
                nc.sync.dma_start(out=w2_t, in_=wm2_d[128 * c16:128 * (c16 + 1), :])
                wm2.append(w2_t)
            mask_t = sb.tile([128, NH * 128], BF16, tag="mask", name="mask_t")
            nc.sync.dma_start(out=mask_t, in_=mask_d)
            idm = sb.tile([128, 128], BF16, tag="idm", name="idm")
            nc.sync.dma_start(out=idm, in_=idm_d)
            onesr = sb.tile([128, 1], F32R, tag="onesr", name="onesr")
            nc.sync.dma_start(out=onesr, in_=ones_d.bitcast(F32R))
            onesb = sb.tile([128, 1], BF16, tag="onesb", name="onesb")
            nc.vector.memset(onesb, 1.0)
            dq_t = sb.tile([128, 8], F32, tag="dq", name="dq_t")
            nc.sync.dma_start(out=dq_t, in_=dq_d)
            pb_t = sb.tile([128, 4], F32, tag="pbt", name="pb_t")
            nc.sync.dma_start(out=pb_t, in_=pb_d)
            d1_t = sb.tile([128, 16], F32, tag="d1t", name="d1_t")
            nc.sync.dma_start(out=d1_t, in_=d1_d)
            b2_t = sb.tile([128, 4], F32, tag="b2t", name="b2_t")
            nc.sync.dma_start(out=b2_t, in_=b2_d)
            dvb = sb.tile([128, C], F32, tag="dvb", name="dvb")
            nc.sync.dma_start(out=dvb, in_=bass.AP(
                tensor=dv_d.tensor, offset=dv_d.offset, ap=[[0, 128], [1, C]]))
            eps_t = sb.tile([1, 1], F32, tag="eps", name="eps_t")
            nc.vector.memset(eps_t, EPS)

            def layernorm(xc, sfx):
                """xc: 4 chunk tiles [128,1024] F32R -> (r_bc, mur_bc)."""
                st = sb.tile([128, 1024], F32, tag="stat", bufs=2,
                             name=f"st_{sfx}")
                for hh in range(2):
                    sl = slice(512 * hh, 512 * (hh + 1))
                    sx = ps.tile([128, 512], F32, tag="sx", bufs=1,
                                 name=f"sx_{sfx}_{hh}")
                    for c4 in range(4):
                        nc.tensor.matmul(sx[0:1, :], onesr, xc[c4][:, sl],
                                         start=(c4 == 0), stop=(c4 == 3))
                    sx2 = ps.tile([128, 512], F32, tag="sx2", bufs=1,
                                  name=f"sx2_{sfx}_{hh}")
                    for c4 in range(4):
                        x2 = sb.tile([128, 512], BF16, tag="zt", bufs=2,
                                     name=f"x2_{sfx}_{hh}_{c4}")
                        nc.vector.tensor_mul(x2, xc[c4][:, sl].bitcast(F32),
                                             xc[c4][:, sl].bitcast(F32))
                        nc.tensor.matmul(sx2[0:1, :], onesb, x2,
                                         start=(c4 == 0), stop=(c4 == 3))
                    # mu2 = (sum(x)/C)^2 straight from psum
                    nc.scalar.activation(st[0:1, sl], sx[0:1, :], AF.Square,
                                         scale=1.0 / C)
                    nc.vector.scalar_tensor_tensor(st[64:65, sl], sx2[0:1, :],
                                                   1.0 / C, st[0:1, sl],
                                                   ALU.mult, ALU.subtract)
                    nc.scalar.activation(st[96:97, sl], st[64:65, sl], AF.Ln,
                                         bias=eps_t)
                    nc.scalar.activation(st[32:33, sl], st[96:97, sl], AF.Exp,
                                         scale=-0.5)
                    # mur = (sum(x)/C) * r ; psum in0 is exempt from the
                    # same-base-partition constraint
                    nc.vector.scalar_tensor_tensor(st[64:65, sl], sx[0:1, :],
                                                   1.0 / C, st[32:33, sl],
                                                   ALU.mult, ALU.mult)
                row = dr.tile([2, 1024], BF16, tag="rt", bufs=2,
                              name=f"row_{sfx}")
                # casting DMAs (gpsimd) write r and mur rows
                nc.gpsimd.dma_start(out=row[0:1, :], in_=st[32:33, :])
                nc.gpsimd.dma_start(out=row[1:2, :], in_=st[64:65, :])
                rm_bc = sb.tile([128, 2048], BF16, tag="bc", bufs=1,
                                name=f"rmbc_{sfx}")
                nc.sync.dma_start(out=rm_bc, in_=bass.AP(
                    tensor=row.tensor, offset=row.offset,
                    ap=[[0, 128], [1, 2048]]))
                return rm_bc[:, 0:1024], rm_bc[:, 1024:2048]

            def z_pass(xc, r_bc, mur_bc, sfx):
                zc = []
                for c4 in range(4):
                    t1 = sb.tile([128, 1024], BF16, tag="zt", bufs=2,
                                 name=f"t1_{sfx}_{c4}")
                    nc.vector.tensor_mul(t1, xc[c4].bitcast(F32), r_bc)
                    z = sb.tile([128, 1024], BF16, tag="z", bufs=8,
                                name=f"z_{sfx}_{c4}")
                    nc.vector.tensor_tensor(out=z, in0=t1, in1=mur_bc,
                                            op=ALU.subtract)
                    zc.append(z)
                return zc

            def load_ln1(img):
                xc = []
                for c4 in range(4):
                    xraw = sb.tile([128, 1024], F32, tag="xraw", bufs=2,
                                   name=f"xr_{img}_{c4}")
                    nc.sync.dma_start(
                        out=xraw,
                        in_=x_d[img, 128 * c4:128 * (c4 + 1), :, :]
                        .rearrange("c h w -> c (h w)"))
                    xt = sb.tile([128, 1024], F32R, tag="xc", bufs=8,
                                 name=f"x_{img}_{c4}")
                    nc.vector.tensor_copy(_ap(xt, 0, WIN8),
                                          _ap(xraw, 0, RAS8).bitcast(F32R))
                    xc.append(xt)
                r_bc, mur_bc = layernorm(xc, f"l1_{img}")
                zc = z_pass(xc, r_bc, mur_bc, f"l1_{img}")
                return xc, zc

            # ---------------- per-image pipeline ----------------
            nxt = load_ln1(0)
            for img in range(BI):
                xc, zc = nxt

                # ---- qkv q/k f-tiles
                qk = {}
                for fi in (0, 4, 1, 5, 2, 6, 3, 7):
                    qkt = sb.tile([128, 1024], BF16, tag="qk", bufs=7,
                                  name=f"qk_{img}_{fi}")
                    for th in range(2):
                        mm = ps.tile([128, 512], F32, tag="mm", bufs=2,
                                     name=f"qkp_{img}_{fi}_{th}")
                        for c4 in range(4):
                            nc.tensor.matmul(
                                mm, wqkv[c4][:, 128 * fi:128 * (fi + 1)],
                                zc[c4][:, 512 * th:512 * (th + 1)],
                                start=(c4 == 0), stop=(c4 == 3))
                        nc.vector.tensor_scalar_add(
                            qkt[:, 512 * th:512 * (th + 1)], mm,
                            dq_t[:, fi:fi + 1])
                    qk[fi] = qkt

                # ---- v (token-major with interleaved ones column)
                vaug = []
                for g in range(NG):
                    mm = ps.tile([128, 512], F32, tag="mm", bufs=2,
                                 name=f"vp_{img}_{g}")
                    for c4 in range(4):
                        nc.tensor.matmul(
                            mm, zc[c4][:, 128 * g:128 * (g + 1)],
                            wqkv[c4][:, 2 * C:3 * C],
                            start=(c4 == 0), stop=(c4 == 3))
                    va = sb.tile([128, 33 * NH], BF16, tag="vaug", bufs=8,
                                 name=f"va_{img}_{g}")
                    nc.vector.memset(_ap(va, 32, [[33, NH]]), 1.0)
                    nc.vector.tensor_tensor(
                        out=_ap(va, 0, [[33, NH], [1, 32]]),
                        in0=_ap(mm, 0, [[32, NH], [1, 32]]),
                        in1=_ap(dvb, 0, [[32, NH], [1, 32]]),
                        op=ALU.add)
                    vaug.append(va)

                # ---- attention, per head-quarter
                atc = [sb.tile([128, 512], BF16, tag="atc", bufs=8,
                               name=f"atc_{img}_{g}") for g in range(NG)]
                for qt in range(4):
                    qh = sb.tile([32, 4 * 1024], BF16, tag="qh", bufs=2,
                                 name=f"qh_{img}_{qt}")
                    kh = sb.tile([32, 4 * 1024], BF16, tag="kh", bufs=1,
                                 name=f"kh_{img}_{qt}")
                    for b4 in range(4):
                        nc.sync.dma_start(
                            out=qh[0:32, 1024 * b4:1024 * (b4 + 1)],
                            in_=qk[qt][32 * b4:32 * (b4 + 1), :])
                        nc.sync.dma_start(
                            out=kh[0:32, 1024 * b4:1024 * (b4 + 1)],
                            in_=qk[4 + qt][32 * b4:32 * (b4 + 1), :])
                    for g in range(NG):
                        stp = ps.tile([128, 512], F32, tag="st", bufs=2,
                                      name=f"stp_{img}_{qt}_{g}")
                        for b4 in range(4):
                            sl = slice(1024 * b4 + 128 * g,
                                       1024 * b4 + 128 * (g + 1))
                            nc.tensor.matmul(
                                stp[:, 128 * b4:128 * (b4 + 1)],
                                kh[0:32, sl], qh[0:32, sl],
                                start=True, stop=True)
                        pt = sb.tile([128, 512], BF16, tag="pt", bufs=2,
                                     name=f"pt_{img}_{qt}_{g}")
                        nc.scalar.activation(pt, stp, AF.Exp)
                        nc.vector.tensor_mul(
                            pt, pt, mask_t[:, 512 * qt:512 * (qt + 1)])
                        av = ps.tile([128, 132], F32, tag="mm", bufs=2,
                                     name=f"av_{img}_{qt}_{g}")
                        for b4 in range(4):
                            h = 4 * qt + b4
                            nc.tensor.matmul(
                                av[:, 33 * b4:33 * (b4 + 1)],
                                pt[:, 128 * b4:128 * (b4 + 1)],
                                vaug[g][:, 33 * h:33 * (h + 1)],
                                start=True, stop=True)
                        rec = sb.tile([128, 4], F32, tag="rec", bufs=2,
                                      name=f"rec_{img}_{qt}_{g}")
                        nc.vector.reciprocal(rec, _ap(av, 32, [[33, 4]]))
                        nc.vector.tensor_tensor(
                            out=_ap(atc[g], 128 * qt, [[32, 4], [1, 32]]),
                            in0=_ap(av, 0, [[33, 4], [1, 32]]),
                            in1=_ap(rec, 0, [[1, 4], [0, 32]]),
                            op=ALU.mult)

                # ---- transpose attention output to channel-major
                actn = []
                for fp in range(4):
                    at = sb.tile([128, 1024], BF16, tag="actn", bufs=4,
                                 name=f"actn_{img}_{fp}")
                    for Q in range(2):
                        tp = ps.tile([128, 512], BF16, tag="av", bufs=2,
                                     name=f"tp_{img}_{fp}_{Q}")
                        for gq in range(4):
                            g = 4 * Q + gq
                            nc.tensor.transpose(
                                tp[:, 128 * gq:128 * (gq + 1)],
                                atc[g][:, 128 * fp:128 * (fp + 1)], idm)
                        nc.scalar.copy(at[:, 512 * Q:512 * (Q + 1)], tp)
                    actn.append(at)

                # ---- proj + residual (in-place xh into xc, window->raster)
                for fo in range(4):
                    for th in range(2):
                        mm = ps.tile([128, 512], F32, tag="mm", bufs=2,
                                     name=f"pj_{img}_{fo}_{th}")
                        for c4 in range(4):
                            nc.tensor.matmul(
                                mm, wproj[c4][:, 128 * fo:128 * (fo + 1)],
                                actn[c4][:, 512 * th:512 * (th + 1)],
                                start=(c4 == 0), stop=(c4 == 3))
                        xap = xc[fo][:, 512 * th:512 * (th + 1)]
                        nc.vector.scalar_tensor_tensor(
                            xap, mm, pb_t[:, fo:fo + 1], xap,
                            ALU.add, ALU.add)

                # prefetch next image's LN1 pipeline under this image's tail
                if img + 1 < BI:
                    nxt = load_ln1(img + 1)

                r2_bc, mur2_bc = layernorm(xc, f"l2_{img}")
                z2c = z_pass(xc, r2_bc, mur2_bc, f"l2_{img}")

                # ---- MLP
                for th in range(2):
                    gel = []
                    for f16 in range(16):
                        mm = ps.tile([128, 512], F32, tag="mm", bufs=2,
                                     name=f"m1_{img}_{th}_{f16}")
                        for c4 in range(4):
                            nc.tensor.matmul(
                                mm, wm1[c4][:, 128 * f16:128 * (f16 + 1)],
                                z2c[c4][:, 512 * th:512 * (th + 1)],
                                start=(c4 == 0), stop=(c4 == 3))
                        gt = sb.tile([128, 512], BF16, tag="gelu", bufs=16,
                                     name=f"g_{img}_{th}_{f16}")
                        nc.scalar.activation(gt, mm, AF.Gelu,
                                             bias=d1_t[:, f16:f16 + 1])
                        gel.append(gt)
                    for fo in range(4):
                        mm2 = ps.tile([128, 512], F32, tag="av", bufs=2,
                                      name=f"m2_{img}_{th}_{fo}")
                        for c16 in range(16):
                            nc.tensor.matmul(
                                mm2, wm2[c16][:, 128 * fo:128 * (fo + 1)],
                                gel[c16], start=(c16 == 0), stop=(c16 == 15))
                        xap = xc[fo][:, 512 * th:512 * (th + 1)]
                        nc.vector.scalar_tensor_tensor(
                            xap, mm2, b2_t[:, fo:fo + 1], xap,
                            ALU.add, ALU.add)

                # ---- store (permute window->raster, then contiguous DMA)
                for c4 in range(4):
                    xo = sb.tile([128, 1024], F32, tag="xraw", bufs=2,
                                 name=f"xo_{img}_{c4}")
                    nc.vector.tensor_copy(_ap(xo, 0, RAS8),
                                          _ap(xc[c4], 0, WIN8).bitcast(F32))
                    nc.sync.dma_start(
                        out=out_d[img, 128 * c4:128 * (c4 + 1), :, :]
                        .rearrange("c h w -> c (h w)"),
                        in_=xo)

    nc.compile()
    return nc


def _prep_weights(inputs):
    """Host-side weight preprocessing (numpy, ~ms)."""
    g1 = np.asarray(inputs["norm1_w"], np.float32)
    b1 = np.asarray(inputs["norm1_b"], np.float32)
    g2 = np.asarray(inputs["norm2_w"], np.float32)
    b2n = np.asarray(inputs["norm2_b"], np.float32)
    wqkv = np.array(inputs["qkv_w"], np.float32)              # [3C, C]
    bqkv = np.array(inputs["qkv_b"], np.float32)
    scale = HD ** -0.5
    wqkv[:C] *= scale
    bqkv = bqkv.copy()
    bqkv[:C] *= scale
    dqkv = wqkv @ b1 + bqkv                                   # [3C]
    wqkvT = (wqkv * g1[None, :]).T                            # [C, 3C]

    wproj = np.asarray(inputs["proj_w"], np.float32)          # [C, C]
    pb = np.asarray(inputs["proj_b"], np.float32)
    wm1 = np.asarray(inputs["mlp_w1"], np.float32)            # [MLP_H, C]
    d1 = wm1 @ b2n + np.asarray(inputs["mlp_b1"], np.float32)
    wm1T = (wm1 * g2[None, :]).T                              # [C, MLP_H]
    wm2 = np.asarray(inputs["mlp_w2"], np.float32)            # [C, MLP_H]
    b2o = np.asarray(inputs["mlp_b2"], np.float32)

    rpb = np.asarray(inputs["rpb_table"], np.float32)         # [(2ws-1)^2, NH]
    rel = _relative_position_index(WS)                        # [N, N] (n, m)
    bias = rpb[rel.reshape(-1)].reshape(N, N, NH)             # [n, m, h]
    eb = np.exp(bias)
    mask = np.zeros((128, NH, 128), np.float32)
    for wdx in range(8):
        # tile entry [k, h, q]: k = 16w + m, q = 16w + n -> eb[n, m, h]
        mask[16 * wdx:16 * (wdx + 1), :, 16 * wdx:16 * (wdx + 1)] = \
            eb.transpose(1, 2, 0)
    mask2d = np.ascontiguousarray(
        mask.reshape(128, NH * 128))

    return {
        "wqkv": np.ascontiguousarray(wqkvT).astype(BF),
        "dq": np.ascontiguousarray(
            dqkv[:2 * C].reshape(8, 128).T).astype(np.float32),
        "dvrow": dqkv[2 * C:].reshape(1, C).astype(np.float32),
        "wproj": np.ascontiguousarray(wproj.T).astype(BF),
        "pb": np.ascontiguousarray(pb.reshape(4, 128).T).astype(np.float32),
        "wm1": np.ascontiguousarray(wm1T).astype(BF),
        "d1": np.ascontiguousarray(d1.reshape(16, 128).T).astype(np.float32),
        "wm2": np.ascontiguousarray(wm2.T).astype(BF),
        "b2": np.ascontiguousarray(b2o.reshape(4, 128).T).astype(np.float32),
        "mask": mask2d.astype(BF),
        "idm": np.eye(128, dtype=BF),
        "onesc": np.ones((128, 1), np.float32),
    }


def get_program():
    if "nc" not in _cache:
        _cache["nc"] = _build_program()
    return _cache["nc"]


def make_in_maps(inputs):
    wmaps = _prep_weights(inputs)
    x_full = np.asarray(inputs["x"], np.float32)
    in_maps = []
    for core in range(NCORES):
        m = dict(wmaps)
        m["x"] = np.ascontiguousarray(x_full[BI * core:BI * (core + 1)])
        in_maps.append(m)
    return in_maps


def kernel(**inputs):
    nc = get_program()
    in_maps = make_in_maps(inputs)
    res = run_bass_kernel_spmd(nc, in_maps, list(range(NCORES)))
    out = np.concatenate([res.results[c]["out"] for c in range(NCORES)],
                         axis=0)
    return out


# revision 23
# speedup vs baseline: 5759.1110x; 1.0312x over previous
"""Swin-style windowed-attention block (LN->W-MSA->residual->LN->MLP->residual)
for TRN2, data-parallel over batch across 8 NeuronCores.

Layout strategy: channels-on-partitions (CT) end to end; x arrives [B,C,H,W]
which is already channel-major per image. LayerNorm stats via ones-matmuls on
the PE (partition-dim sums), per-token scale/shift via DRAM-round-trip
partition broadcasts. Attention computes S-transposed (keys on partitions)
from per-head partition-0 tiles produced by an SBUF->SBUF DMA rearrangement;
softmax without max-subtraction (scores are small by construction); the
relative-position bias and the block-diagonal window mask are folded into one
host-precomputed multiplicative exp(bias) mask; row-sums ride along the AV
matmul as a ones-column of the value matrix; AV uses P~ as the stationary
operand giving token-major outputs that a cheap PE transpose returns to CT.
"""
import sys
import numpy as np
import ml_dtypes

sys.path.insert(0, "/opt/trn_rl_repo")

import concourse.bass as bass
import concourse.bacc as bacc
import concourse.tile as tile
from concourse import mybir
from concourse.bass_utils import run_bass_kernel_spmd

F32 = mybir.dt.float32
F32R = mybir.dt.float32r
BF16 = mybir.dt.bfloat16
AF = mybir.ActivationFunctionType
ALU = mybir.AluOpType
BF = ml_dtypes.bfloat16

# problem constants (hardcoded per the task contract)
B, C, H, W = 32, 512, 32, 32
NH, WS = 16, 4
HD = C // NH            # 32
N = WS * WS             # 16 tokens per window
EPS = 1e-5
MLP_H = 4 * C           # 2048
NCORES = 8
BI = B // NCORES        # images per core = 4
T = H * W               # tokens per image = 1024
NG = T // 128           # 128-token groups per image = 8

_cache = {}


def _relative_position_index(ws):
    coords = np.stack(np.meshgrid(np.arange(ws), np.arange(ws), indexing="ij"))
    cf = coords.reshape(2, -1)
    rel = cf[:, :, None] - cf[:, None, :]
    rel = rel.transpose(1, 2, 0).astype(np.int64)
    rel[:, :, 0] += ws - 1
    rel[:, :, 1] += ws - 1
    rel[:, :, 0] *= 2 * ws - 1
    return rel.sum(-1)


def _ap(t, off, dims):
    return bass.AP(tensor=t.tensor, offset=t.offset + off,
                   ap=[t.ap[0]] + [list(d) for d in dims])

# window permutation: window-ordered col = 128g + 16ww + 4i + j
#                     raster col         = 128g + 32i + 4ww + j
WIN4 = [[128, 4], [16, 8], [4, 4], [1, 4]]    # half-image (4 groups)
RAS4 = [[128, 4], [4, 8], [32, 4], [1, 4]]
WIN8 = [[128, 8], [16, 8], [4, 4], [1, 4]]    # full image (8 groups)
RAS8 = [[128, 8], [4, 8], [32, 4], [1, 4]]


def _build_program():
    nc = bacc.Bacc("TRN2", target_bir_lowering=False, debug=False,
                   enable_asserts=True, num_devices=NCORES)

    def din(name, shape, dt):
        return nc.dram_tensor(name, shape, dt, kind="ExternalInput").ap()

    x_d = din("x", (BI, C, H, W), F32)
    wqkv_d = din("wqkv", (C, 3 * C), BF16)        # [c, f] = diag(g1) @ Wqkv.T
    dq_d = din("dq", (128, 8), F32)               # q/k bias, col per f-tile
    dv_d = din("dvrow", (1, C), F32)              # v bias row (bcast source)
    wproj_d = din("wproj", (C, C), BF16)
    pb_d = din("pb", (128, 4), F32)
    wm1_d = din("wm1", (C, MLP_H), BF16)
    d1_d = din("d1", (128, 16), F32)
    wm2_d = din("wm2", (MLP_H, C), BF16)
    b2_d = din("b2", (128, 4), F32)
    mask_d = din("mask", (128, NH * 128), BF16)
    idm_d = din("idm", (128, 128), BF16)
    ones_d = din("onesc", (128, 1), F32)

    out_d = nc.dram_tensor("out", (BI, C, H, W), F32, kind="ExternalOutput").ap()

    with tile.TileContext(nc) as tc:
        with tc.tile_pool(name="sb", bufs=1) as sb, \
             tc.tile_pool(name="dr", bufs=1, space="DRAM") as dr, \
             tc.tile_pool(name="ps", bufs=1, space="PSUM") as ps:

            # ---------------- resident constants ----------------
            wqkv = []
            for c4 in range(4):
                wq_t = sb.tile([128, 3 * C], BF16, tag="wqkv", bufs=4,
                               name=f"wqkv{c4}")
                nc.sync.dma_start(out=wq_t, in_=wqkv_d[128 * c4:128 * (c4 + 1), :])
                wqkv.append(wq_t)
            wproj = []
            for c4 in range(4):
                wp_t = sb.tile([128, C], BF16, tag="wproj", bufs=4,
                               name=f"wproj{c4}")
                nc.sync.dma_start(out=wp_t, in_=wproj_d[128 * c4:128 * (c4 + 1), :])
                wproj.append(wp_t)
            wm1 = []
            for c4 in range(4):
                w1_t = sb.tile([128, MLP_H], BF16, tag="wm1", bufs=4,
                               name=f"wm1{c4}")
                nc.sync.dma_start(out=w1_t, in_=wm1_d[128 * c4:128 * (c4 + 1), :])
                wm1.append(w1_t)
            wm2 = []
            for c16 in range(16):
                w2_t = sb.tile([128, C], BF16, tag="wm2", bufs=16,
                               name=f"wm2{c16}")
                nc.sync.dma_start(out=w2_t, in_=wm2_d[128 * c16:128 * (c16 + 1), :])
                wm2.append(w2_t)
            mask_t = sb.tile([128, NH * 128], BF16, tag="mask", name="mask_t")
            nc.sync.dma_start(out=mask_t, in_=mask_d)
            idm = sb.tile([128, 128], BF16, tag="idm", name="idm")
            nc.sync.dma_start(out=idm, in_=idm_d)
            onesr = sb.tile([128, 1], F32R, tag="onesr", name="onesr")
            nc.sync.dma_start(out=onesr, in_=ones_d.bitcast(F32R))
            onesb = sb.tile([128, 1], BF16, tag="onesb", name="onesb")
            nc.vector.memset(onesb, 1.0)
            dq_t = sb.tile([128, 8], F32, tag="dq", name="dq_t")
            nc.sync.dma_start(out=dq_t, in_=dq_d)
            pb_t = sb.tile([128, 4], F32, tag="pbt", name="pb_t")
            nc.sync.dma_start(out=pb_t, in_=pb_d)
            d1_t = sb.tile([128, 16], F32, tag="d1t", name="d1_t")
            nc.sync.dma_start(out=d1_t, in_=d1_d)
            b2_t = sb.tile([128, 4], F32, tag="b2t", name="b2_t")
            nc.sync.dma_start(out=b2_t, in_=b2_d)
            dvb = sb.tile([128, C], F32, tag="dvb", name="dvb")
            nc.sync.dma_start(out=dvb, in_=bass.AP(
                tensor=dv_d.tensor, offset=dv_d.offset, ap=[[0, 128], [1, C]]))
            eps_t = sb.tile([1, 1], F32, tag="eps", name="eps_t")
            nc.vector.memset(eps_t, EPS)

            def layernorm(xc, sfx):
                """xc: 4 chunk tiles [128,1024] F32R -> (r_bc, mur_bc)."""
                st = sb.tile([128, 1024], F32, tag="stat", bufs=2,
                             name=f"st_{sfx}")
                for hh in range(2):
                    sl = slice(512 * hh, 512 * (hh + 1))
                    sx = ps.tile([128, 512], F32, tag="sx", bufs=1,
                                 name=f"sx_{sfx}_{hh}")
                    for c4 in range(4):
                        nc.tensor.matmul(sx[0:1, :], onesr, xc[c4][:, sl],
                                         start=(c4 == 0), stop=(c4 == 3))
                    sx2 = ps.tile([128, 512], F32, tag="sx2", bufs=1,
                                  name=f"sx2_{sfx}_{hh}")
                    for c4 in range(4):
                        x2 = sb.tile([128, 512], BF16, tag="zt", bufs=2,
                                     name=f"x2_{sfx}_{hh}_{c4}")
                        nc.vector.tensor_mul(x2, xc[c4][:, sl].bitcast(F32),
                                             xc[c4][:, sl].bitcast(F32))
                        nc.tensor.matmul(sx2[0:1, :], onesb, x2,
                                         start=(c4 == 0), stop=(c4 == 3))
                    # mu2 = (sum(x)/C)^2 straight from psum
                    nc.scalar.activation(st[0:1, sl], sx[0:1, :], AF.Square,
                                         scale=1.0 / C)
                    nc.vector.scalar_tensor_tensor(st[64:65, sl], sx2[0:1, :],
                                                   1.0 / C, st[0:1, sl],
                                                   ALU.mult, ALU.subtract)
                    nc.scalar.activation(st[96:97, sl], st[64:65, sl], AF.Ln,
                                         bias=eps_t)
                    nc.scalar.activation(st[32:33, sl], st[96:97, sl], AF.Exp,
                                         scale=-0.5)
                    # mur = (sum(x)/C) * r ; psum in0 is exempt from the
                    # same-base-partition constraint
                    nc.vector.scalar_tensor_tensor(st[64:65, sl], sx[0:1, :],
                                                   1.0 / C, st[32:33, sl],
                                                   ALU.mult, ALU.mult)
                row = dr.tile([2, 1024], BF16, tag="rt", bufs=2,
                              name=f"row_{sfx}")
                # casting DMAs (gpsimd) write r and mur rows
                nc.gpsimd.dma_start(out=row[0:1, :], in_=st[32:33, :])
                nc.gpsimd.dma_start(out=row[1:2, :], in_=st[64:65, :])
                rm_bc = sb.tile([128, 2048], BF16, tag="bc", bufs=1,
                                name=f"rmbc_{sfx}")
                nc.sync.dma_start(out=rm_bc, in_=bass.AP(
                    tensor=row.tensor, offset=row.offset,
                    ap=[[0, 128], [1, 2048]]))
                return rm_bc[:, 0:1024], rm_bc[:, 1024:2048]

            def z_pass(xc, r_bc, mur_bc, sfx):
                zc = []
                for c4 in range(4):
                    t1 = sb.tile([128, 1024], BF16, tag="zt", bufs=2,
                                 name=f"t1_{sfx}_{c4}")
                    nc.vector.tensor_mul(t1, xc[c4].bitcast(F32), r_bc)
                    z = sb.tile([128, 1024], BF16, tag="z", bufs=8,
                                name=f"z_{sfx}_{c4}")
                    nc.vector.tensor_tensor(out=z, in0=t1, in1=mur_bc,
                                            op=ALU.subtract)
                    zc.append(z)
                return zc

            def load_ln1(img):
                xc = []
                for c4 in range(4):
                    xraw = sb.tile([128, 1024], F32, tag="xraw", bufs=2,
                                   name=f"xr_{img}_{c4}")
                    nc.sync.dma_start(
                        out=xraw,
                        in_=x_d[img, 128 * c4:128 * (c4 + 1), :, :]
                        .rearrange("c h w -> c (h w)"))
                    xt = sb.tile([128, 1024], F32R, tag="xc", bufs=8,
                                 name=f"x_{img}_{c4}")
                    nc.vector.tensor_copy(_ap(xt, 0, WIN8),
                                          _ap(xraw, 0, RAS8).bitcast(F32R))
                    xc.append(xt)
                r_bc, mur_bc = layernorm(xc, f"l1_{img}")
                zc = z_pass(xc, r_bc, mur_bc, f"l1_{img}")
                return xc, zc


            def qkv_v(zc, img):
                # ---- qkv q/k f-tiles
                qk = {}
                for fi in (0, 4, 1, 5, 2, 6, 3, 7):
                    qkt = sb.tile([128, 1024], BF16, tag="qk", bufs=7,
                                  name=f"qk_{img}_{fi}")
                    for th in range(2):
                        mm = ps.tile([128, 512], F32, tag="mm", bufs=2,
                                     name=f"qkp_{img}_{fi}_{th}")
                        for c4 in range(4):
                            nc.tensor.matmul(
                                mm, wqkv[c4][:, 128 * fi:128 * (fi + 1)],
                                zc[c4][:, 512 * th:512 * (th + 1)],
                                start=(c4 == 0), stop=(c4 == 3))
                        nc.vector.tensor_scalar_add(
                            qkt[:, 512 * th:512 * (th + 1)], mm,
                            dq_t[:, fi:fi + 1])
                    qk[fi] = qkt

                # ---- v (token-major with interleaved ones column)
                vaug = []
                for g in range(NG):
                    mm = ps.tile([128, 512], F32, tag="mm", bufs=2,
                                 name=f"vp_{img}_{g}")
                    for c4 in range(4):
                        nc.tensor.matmul(
                            mm, zc[c4][:, 128 * g:128 * (g + 1)],
                            wqkv[c4][:, 2 * C:3 * C],
                            start=(c4 == 0), stop=(c4 == 3))
                    va = sb.tile([128, 33 * NH], BF16, tag="vaug", bufs=8,
                                 name=f"va_{img}_{g}")
                    nc.vector.memset(_ap(va, 32, [[33, NH]]), 1.0)
                    nc.vector.tensor_tensor(
                        out=_ap(va, 0, [[33, NH], [1, 32]]),
                        in0=_ap(mm, 0, [[32, NH], [1, 32]]),
                        in1=_ap(dvb, 0, [[32, NH], [1, 32]]),
                        op=ALU.add)
                    vaug.append(va)
                return qk, vaug

            # ---------------- per-image pipeline ----------------
            _xc0, _zc0 = load_ln1(0)
            _qk0, _va0 = qkv_v(_zc0, 0)
            nxt = (_xc0, _qk0, _va0)
            for img in range(BI):
                xc, qk, vaug = nxt

                # ---- attention, per head-quarter
                atc = [sb.tile([128, 512], BF16, tag="atc", bufs=8,
                               name=f"atc_{img}_{g}") for g in range(NG)]
                for qt in range(4):
                    qh = sb.tile([32, 4 * 1024], BF16, tag="qh", bufs=2,
                                 name=f"qh_{img}_{qt}")
                    kh = sb.tile([32, 4 * 1024], BF16, tag="kh", bufs=1,
                                 name=f"kh_{img}_{qt}")
                    for b4 in range(4):
                        nc.sync.dma_start(
                            out=qh[0:32, 1024 * b4:1024 * (b4 + 1)],
                            in_=qk[qt][32 * b4:32 * (b4 + 1), :])
                        nc.sync.dma_start(
                            out=kh[0:32, 1024 * b4:1024 * (b4 + 1)],
                            in_=qk[4 + qt][32 * b4:32 * (b4 + 1), :])
                    for g in range(NG):
                        stp = ps.tile([128, 512], F32, tag="st", bufs=2,
                                      name=f"stp_{img}_{qt}_{g}")
                        for b4 in range(4):
                            sl = slice(1024 * b4 + 128 * g,
                                       1024 * b4 + 128 * (g + 1))
                            nc.tensor.matmul(
                                stp[:, 128 * b4:128 * (b4 + 1)],
                                kh[0:32, sl], qh[0:32, sl],
                                start=True, stop=True)
                        pt = sb.tile([128, 512], BF16, tag="pt", bufs=2,
                                     name=f"pt_{img}_{qt}_{g}")
                        nc.scalar.activation(pt, stp, AF.Exp)
                        nc.vector.tensor_mul(
                            pt, pt, mask_t[:, 512 * qt:512 * (qt + 1)])
                        av = ps.tile([128, 132], F32, tag="mm", bufs=2,
                                     name=f"av_{img}_{qt}_{g}")
                        for b4 in range(4):
                            h = 4 * qt + b4
                            nc.tensor.matmul(
                                av[:, 33 * b4:33 * (b4 + 1)],
                                pt[:, 128 * b4:128 * (b4 + 1)],
                                vaug[g][:, 33 * h:33 * (h + 1)],
                                start=True, stop=True)
                        rec = sb.tile([128, 4], F32, tag="rec", bufs=2,
                                      name=f"rec_{img}_{qt}_{g}")
                        nc.vector.reciprocal(rec, _ap(av, 32, [[33, 4]]))
                        nc.vector.tensor_tensor(
                            out=_ap(atc[g], 128 * qt, [[32, 4], [1, 32]]),
                            in0=_ap(av, 0, [[33, 4], [1, 32]]),
                            in1=_ap(rec, 0, [[1, 4], [0, 32]]),
                            op=ALU.mult)

                # ---- transpose attention output to channel-major
                actn = []
                for fp in range(4):
                    at = sb.tile([128, 1024], BF16, tag="actn", bufs=4,
                                 name=f"actn_{img}_{fp}")
                    for Q in range(2):
                        tp = ps.tile([128, 512], BF16, tag="av", bufs=2,
                                     name=f"tp_{img}_{fp}_{Q}")
                        for gq in range(4):
                            g = 4 * Q + gq
                            nc.tensor.transpose(
                                tp[:, 128 * gq:128 * (gq + 1)],
                                atc[g][:, 128 * fp:128 * (fp + 1)], idm)
                        nc.scalar.copy(at[:, 512 * Q:512 * (Q + 1)], tp)
                    actn.append(at)

                # ---- proj + residual (in-place xh into xc, window->raster)
                for fo in range(4):
                    for th in range(2):
                        mm = ps.tile([128, 512], F32, tag="mm", bufs=2,
                                     name=f"pj_{img}_{fo}_{th}")
                        for c4 in range(4):
                            nc.tensor.matmul(
                                mm, wproj[c4][:, 128 * fo:128 * (fo + 1)],
                                actn[c4][:, 512 * th:512 * (th + 1)],
                                start=(c4 == 0), stop=(c4 == 3))
                        xap = xc[fo][:, 512 * th:512 * (th + 1)]
                        nc.vector.scalar_tensor_tensor(
                            xap, mm, pb_t[:, fo:fo + 1], xap,
                            ALU.add, ALU.add)

                # prefetch next image's LN1 + qkv/v under this image's tail
                if img + 1 < BI:
                    _xcn, _zcn = load_ln1(img + 1)
                    _qkn, _van = qkv_v(_zcn, img + 1)
                    nxt = (_xcn, _qkn, _van)

                r2_bc, mur2_bc = layernorm(xc, f"l2_{img}")
                z2c = z_pass(xc, r2_bc, mur2_bc, f"l2_{img}")

                # ---- MLP
                for th in range(2):
                    gel = []
                    for f16 in range(16):
                        mm = ps.tile([128, 512], F32, tag="mm", bufs=2,
                                     name=f"m1_{img}_{th}_{f16}")
                        for c4 in range(4):
                            nc.tensor.matmul(
                                mm, wm1[c4][:, 128 * f16:128 * (f16 + 1)],
                                z2c[c4][:, 512 * th:512 * (th + 1)],
                                start=(c4 == 0), stop=(c4 == 3))
                        gt = sb.tile([128, 512], BF16, tag="gelu", bufs=16,
                                     name=f"g_{img}_{th}_{f16}")
                        nc.scalar.activation(gt, mm, AF.Gelu,
                                             bias=d1_t[:, f16:f16 + 1])
                        gel.append(gt)
                    for fo in range(4):
                        mm2 = ps.tile([128, 512], F32, tag="av", bufs=2,
                                      name=f"m2_{img}_{th}_{fo}")
                        for c16 in range(16):
                            nc.tensor.matmul(
                                mm2, wm2[c16][:, 128 * fo:128 * (fo + 1)],
                                gel[c16], start=(c16 == 0), stop=(c16 == 15))
                        xap = xc[fo][:, 512 * th:512 * (th + 1)]
                        nc.vector.scalar_tensor_tensor(
                            xap, mm2, b2_t[:, fo:fo + 1], xap,
                            ALU.add, ALU.add)

                # ---- store (permute window->raster, then contiguous DMA)
                for c4 in range(4):
                    xo = sb.tile([128, 1024], F32, tag="xraw", bufs=2,
                                 name=f"xo_{img}_{c4}")
                    nc.vector.tensor_copy(_ap(xo, 0, RAS8),
                                          _ap(xc[c4], 0, WIN8).bitcast(F32))
                    nc.sync.dma_start(
                        out=out_d[img, 128 * c4:128 * (c4 + 1), :, :]
                        .rearrange("c h w -> c (h w)"),
                        in_=xo)

    nc.compile()
    return nc


def _prep_weights(inputs):
    """Host-side weight preprocessing (numpy, ~ms)."""
    g1 = np.asarray(inputs["norm1_w"], np.float32)
    b1 = np.asarray(inputs["norm1_b"], np.float32)
    g2 = np.asarray(inputs["norm2_w"], np.float32)
    b2n = np.asarray(inputs["norm2_b"], np.float32)
    wqkv = np.array(inputs["qkv_w"], np.float32)              # [3C, C]
    bqkv = np.array(inputs["qkv_b"], np.float32)
    scale = HD ** -0.5
    wqkv[:C] *= scale
    bqkv = bqkv.copy()
    bqkv[:C] *= scale
    dqkv = wqkv @ b1 + bqkv                                   # [3C]
    wqkvT = (wqkv * g1[None, :]).T                            # [C, 3C]

    wproj = np.asarray(inputs["proj_w"], np.float32)          # [C, C]
    pb = np.asarray(inputs["proj_b"], np.float32)
    wm1 = np.asarray(inputs["mlp_w1"], np.float32)            # [MLP_H, C]
    d1 = wm1 @ b2n + np.asarray(inputs["mlp_b1"], np.float32)
    wm1T = (wm1 * g2[None, :]).T                              # [C, MLP_H]
    wm2 = np.asarray(inputs["mlp_w2"], np.float32)            # [C, MLP_H]
    b2o = np.asarray(inputs["mlp_b2"], np.float32)

    rpb = np.asarray(inputs["rpb_table"], np.float32)         # [(2ws-1)^2, NH]
    rel = _relative_position_index(WS)                        # [N, N] (n, m)
    bias = rpb[rel.reshape(-1)].reshape(N, N, NH)             # [n, m, h]
    eb = np.exp(bias)
    mask = np.zeros((128, NH, 128), np.float32)
    for wdx in range(8):
        # tile entry [k, h, q]: k = 16w + m, q = 16w + n -> eb[n, m, h]
        mask[16 * wdx:16 * (wdx + 1), :, 16 * wdx:16 * (wdx + 1)] = \
            eb.transpose(1, 2, 0)
    mask2d = np.ascontiguousarray(
        mask.reshape(128, NH * 128))

    return {
        "wqkv": np.ascontiguousarray(wqkvT).astype(BF),
        "dq": np.ascontiguousarray(
            dqkv[:2 * C].reshape(8, 128).T).astype(np.float32),
        "dvrow": dqkv[2 * C:].reshape(1, C).astype(np.float32),
        "wproj": np.ascontiguousarray(wproj.T).astype(BF),
        "pb": np.ascontiguousarray(pb.reshape(4, 128).T).astype(np.float32),
        "wm1": np.ascontiguousarray(wm1T).astype(BF),
        "d1": np.ascontiguousarray(d1.reshape(16, 128).T).astype(np.float32),
        "wm2": np.ascontiguousarray(wm2.T).astype(BF),
        "b2": np.ascontiguousarray(b2o.reshape(4, 128).T).astype(np.float32),
        "mask": mask2d.astype(BF),
        "idm": np.eye(128, dtype=BF),
        "onesc": np.ones((128, 1), np.float32),
    }


def get_program():
    if "nc" not in _cache:
        _cache["nc"] = _build_program()
    return _cache["nc"]


def make_in_maps(inputs):
    wmaps = _prep_weights(inputs)
    x_full = np.asarray(inputs["x"], np.float32)
    in_maps = []
    for core in range(NCORES):
        m = dict(wmaps)
        m["x"] = np.ascontiguousarray(x_full[BI * core:BI * (core + 1)])
        in_maps.append(m)
    return in_maps


def kernel(**inputs):
    nc = get_program()
    in_maps = make_in_maps(inputs)
    res = run_bass_kernel_spmd(nc, in_maps, list(range(NCORES)))
    out = np.concatenate([res.results[c]["out"] for c in range(NCORES)],
                         axis=0)
    return out


# revision 31
# speedup vs baseline: 5818.9657x; 1.0104x over previous
"""Swin-style windowed-attention block (LN->W-MSA->residual->LN->MLP->residual)
for TRN2, data-parallel over batch across 8 NeuronCores.

Layout strategy: channels-on-partitions (CT) end to end; x arrives [B,C,H,W]
which is already channel-major per image. LayerNorm stats via ones-matmuls on
the PE (partition-dim sums), per-token scale/shift via DRAM-round-trip
partition broadcasts. Attention computes S-transposed (keys on partitions)
from per-head partition-0 tiles produced by an SBUF->SBUF DMA rearrangement;
softmax without max-subtraction (scores are small by construction); the
relative-position bias and the block-diagonal window mask are folded into one
host-precomputed multiplicative exp(bias) mask; row-sums ride along the AV
matmul as a ones-column of the value matrix; AV uses P~ as the stationary
operand giving token-major outputs that a cheap PE transpose returns to CT.
"""
import sys
import numpy as np
import ml_dtypes

sys.path.insert(0, "/opt/trn_rl_repo")

import concourse.bass as bass
import concourse.bacc as bacc
import concourse.tile as tile
from concourse import mybir
from concourse.bass_utils import run_bass_kernel_spmd

F32 = mybir.dt.float32
F32R = mybir.dt.float32r
BF16 = mybir.dt.bfloat16
AF = mybir.ActivationFunctionType
ALU = mybir.AluOpType
BF = ml_dtypes.bfloat16

# problem constants (hardcoded per the task contract)
B, C, H, W = 32, 512, 32, 32
NH, WS = 16, 4
HD = C // NH            # 32
N = WS * WS             # 16 tokens per window
EPS = 1e-5
MLP_H = 4 * C           # 2048
NCORES = 8
BI = B // NCORES        # images per core = 4
T = H * W               # tokens per image = 1024
NG = T // 128           # 128-token groups per image = 8

_cache = {}


def _relative_position_index(ws):
    coords = np.stack(np.meshgrid(np.arange(ws), np.arange(ws), indexing="ij"))
    cf = coords.reshape(2, -1)
    rel = cf[:, :, None] - cf[:, None, :]
    rel = rel.transpose(1, 2, 0).astype(np.int64)
    rel[:, :, 0] += ws - 1
    rel[:, :, 1] += ws - 1
    rel[:, :, 0] *= 2 * ws - 1
    return rel.sum(-1)


def _ap(t, off, dims):
    return bass.AP(tensor=t.tensor, offset=t.offset + off,
                   ap=[t.ap[0]] + [list(d) for d in dims])

# window permutation: window-ordered col = 128g + 16ww + 4i + j
#                     raster col         = 128g + 32i + 4ww + j
WIN4 = [[128, 4], [16, 8], [4, 4], [1, 4]]    # half-image (4 groups)
RAS4 = [[128, 4], [4, 8], [32, 4], [1, 4]]
WIN8 = [[128, 8], [16, 8], [4, 4], [1, 4]]    # full image (8 groups)
RAS8 = [[128, 8], [4, 8], [32, 4], [1, 4]]


def _build_program():
    nc = bacc.Bacc("TRN2", target_bir_lowering=False, debug=False,
                   enable_asserts=True, num_devices=NCORES)

    def din(name, shape, dt):
        return nc.dram_tensor(name, shape, dt, kind="ExternalInput").ap()

    x_d = din("x", (BI, C, H, W), F32)
    wqkv_d = din("wqkv", (C, 3 * C), BF16)        # [c, f] = diag(g1) @ Wqkv.T
    dq_d = din("dq", (128, 8), F32)               # q/k bias, col per f-tile
    dv_d = din("dvrow", (1, C), F32)              # v bias row (bcast source)
    wproj_d = din("wproj", (C, C), BF16)
    pb_d = din("pb", (128, 4), F32)
    wm1_d = din("wm1", (C, MLP_H), BF16)
    d1_d = din("d1", (128, 16), F32)
    wm2_d = din("wm2", (MLP_H, C), BF16)
    b2_d = din("b2", (128, 4), F32)
    mask_d = din("mask", (128, NH * 128), BF16)
    idm_d = din("idm", (128, 128), BF16)
    ones_d = din("onesc", (128, 1), F32)

    out_d = nc.dram_tensor("out", (BI, C, H, W), F32, kind="ExternalOutput").ap()

    with tile.TileContext(nc) as tc:
        with tc.tile_pool(name="sb", bufs=1) as sb, \
             tc.tile_pool(name="dr", bufs=1, space="DRAM") as dr, \
             tc.tile_pool(name="ps", bufs=1, space="PSUM") as ps:

            # ---------------- resident constants ----------------
            wqkv = []
            for c4 in range(4):
                wq_t = sb.tile([128, 3 * C], BF16, tag="wqkv", bufs=4,
                               name=f"wqkv{c4}")
                nc.sync.dma_start(out=wq_t, in_=wqkv_d[128 * c4:128 * (c4 + 1), :])
                wqkv.append(wq_t)
            wproj = []
            for c4 in range(4):
                wp_t = sb.tile([128, C], BF16, tag="wproj", bufs=4,
                               name=f"wproj{c4}")
                nc.sync.dma_start(out=wp_t, in_=wproj_d[128 * c4:128 * (c4 + 1), :])
                wproj.append(wp_t)
            wm1 = []
            for c4 in range(4):
                w1_t = sb.tile([128, MLP_H], BF16, tag="wm1", bufs=4,
                               name=f"wm1{c4}")
                nc.sync.dma_start(out=w1_t, in_=wm1_d[128 * c4:128 * (c4 + 1), :])
                wm1.append(w1_t)
            wm2 = []
            for c16 in range(16):
                w2_t = sb.tile([128, C], BF16, tag="wm2", bufs=16,
                               name=f"wm2{c16}")
                nc.sync.dma_start(out=w2_t, in_=wm2_d[128 * c16:128 * (c16 + 1), :])
                wm2.append(w2_t)
            mask_t = sb.tile([128, NH * 128], BF16, tag="mask", name="mask_t")
            nc.sync.dma_start(out=mask_t, in_=mask_d)
            idm = sb.tile([128, 128], BF16, tag="idm", name="idm")
            nc.sync.dma_start(out=idm, in_=idm_d)
            onesr = sb.tile([128, 1], F32R, tag="onesr", name="onesr")
            nc.sync.dma_start(out=onesr, in_=ones_d.bitcast(F32R))
            onesb = sb.tile([128, 1], BF16, tag="onesb", name="onesb")
            nc.vector.memset(onesb, 1.0)
            dq_t = sb.tile([128, 8], F32, tag="dq", name="dq_t")
            nc.sync.dma_start(out=dq_t, in_=dq_d)
            pb_t = sb.tile([128, 4], F32, tag="pbt", name="pb_t")
            nc.sync.dma_start(out=pb_t, in_=pb_d)
            d1_t = sb.tile([128, 16], F32, tag="d1t", name="d1_t")
            nc.sync.dma_start(out=d1_t, in_=d1_d)
            b2_t = sb.tile([128, 4], F32, tag="b2t", name="b2_t")
            nc.sync.dma_start(out=b2_t, in_=b2_d)
            dvb = sb.tile([128, C], F32, tag="dvb", name="dvb")
            nc.sync.dma_start(out=dvb, in_=bass.AP(
                tensor=dv_d.tensor, offset=dv_d.offset, ap=[[0, 128], [1, C]]))
            eps_t = sb.tile([1, 1], F32, tag="eps", name="eps_t")
            nc.vector.memset(eps_t, EPS)

            def layernorm(xc, sfx):
                """xc: 4 chunk tiles [128,1024] F32R -> (r_bc, mur_bc)."""
                st = sb.tile([128, 1024], F32, tag="stat", bufs=2,
                             name=f"st_{sfx}")
                for hh in range(2):
                    sl = slice(512 * hh, 512 * (hh + 1))
                    sx = ps.tile([128, 512], F32, tag="sx", bufs=1,
                                 name=f"sx_{sfx}_{hh}")
                    for c4 in range(4):
                        nc.tensor.matmul(sx[0:1, :], onesr, xc[c4][:, sl],
                                         start=(c4 == 0), stop=(c4 == 3))
                    sx2 = ps.tile([128, 512], F32, tag="sx2", bufs=1,
                                  name=f"sx2_{sfx}_{hh}")
                    for c4 in range(4):
                        x2 = sb.tile([128, 512], BF16, tag="zt", bufs=2,
                                     name=f"x2_{sfx}_{hh}_{c4}")
                        nc.vector.tensor_mul(x2, xc[c4][:, sl].bitcast(F32),
                                             xc[c4][:, sl].bitcast(F32))
                        nc.tensor.matmul(sx2[0:1, :], onesb, x2,
                                         start=(c4 == 0), stop=(c4 == 3))
                    # mu2 = (sum(x)/C)^2 straight from psum
                    nc.scalar.activation(st[0:1, sl], sx[0:1, :], AF.Square,
                                         scale=1.0 / C)
                    nc.vector.scalar_tensor_tensor(st[64:65, sl], sx2[0:1, :],
                                                   1.0 / C, st[0:1, sl],
                                                   ALU.mult, ALU.subtract)
                    nc.scalar.activation(st[96:97, sl], st[64:65, sl], AF.Ln,
                                         bias=eps_t)
                    nc.scalar.activation(st[32:33, sl], st[96:97, sl], AF.Exp,
                                         scale=-0.5)
                    # mur = (sum(x)/C) * r ; psum in0 is exempt from the
                    # same-base-partition constraint
                    nc.vector.scalar_tensor_tensor(st[64:65, sl], sx[0:1, :],
                                                   1.0 / C, st[32:33, sl],
                                                   ALU.mult, ALU.mult)
                row = dr.tile([2, 1024], BF16, tag="rt", bufs=2,
                              name=f"row_{sfx}")
                # casting DMAs (gpsimd) write r and mur rows
                nc.gpsimd.dma_start(out=row[0:1, :], in_=st[32:33, :])
                nc.gpsimd.dma_start(out=row[1:2, :], in_=st[64:65, :])
                rm_bc = sb.tile([128, 2048], BF16, tag="bc", bufs=1,
                                name=f"rmbc_{sfx}")
                nc.sync.dma_start(out=rm_bc, in_=bass.AP(
                    tensor=row.tensor, offset=row.offset,
                    ap=[[0, 128], [1, 2048]]))
                return rm_bc[:, 0:1024], rm_bc[:, 1024:2048]

            def z_pass(xc, r_bc, mur_bc, sfx):
                zc = []
                for c4 in range(4):
                    t1 = sb.tile([128, 1024], BF16, tag="zt", bufs=2,
                                 name=f"t1_{sfx}_{c4}")
                    nc.vector.tensor_mul(t1, xc[c4].bitcast(F32), r_bc)
                    z = sb.tile([128, 1024], BF16, tag="z", bufs=8,
                                name=f"z_{sfx}_{c4}")
                    nc.vector.tensor_tensor(out=z, in0=t1, in1=mur_bc,
                                            op=ALU.subtract)
                    zc.append(z)
                return zc

            def load_ln1(img):
                xc = []
                for c4 in range(4):
                    xraw = sb.tile([128, 1024], F32, tag="xraw", bufs=2,
                                   name=f"xr_{img}_{c4}")
                    nc.sync.dma_start(
                        out=xraw,
                        in_=x_d[img, 128 * c4:128 * (c4 + 1), :, :]
                        .rearrange("c h w -> c (h w)"))
                    xt = sb.tile([128, 1024], F32R, tag="xc", bufs=8,
                                 name=f"x_{img}_{c4}")
                    nc.vector.tensor_copy(_ap(xt, 0, WIN8),
                                          _ap(xraw, 0, RAS8).bitcast(F32R))
                    xc.append(xt)
                r_bc, mur_bc = layernorm(xc, f"l1_{img}")
                zc = z_pass(xc, r_bc, mur_bc, f"l1_{img}")
                return xc, zc


            def qkv_v(zc, img):
                # ---- qkv q/k f-tiles
                qk = {}
                for fi in (0, 4, 1, 5, 2, 6, 3, 7):
                    qkt = sb.tile([128, 1024], BF16, tag="qk", bufs=7,
                                  name=f"qk_{img}_{fi}")
                    for th in range(2):
                        mm = ps.tile([128, 512], F32, tag="mm", bufs=2,
                                     name=f"qkp_{img}_{fi}_{th}")
                        for c4 in range(4):
                            nc.tensor.matmul(
                                mm, wqkv[c4][:, 128 * fi:128 * (fi + 1)],
                                zc[c4][:, 512 * th:512 * (th + 1)],
                                start=(c4 == 0), stop=(c4 == 3))
                        nc.vector.tensor_scalar_add(
                            qkt[:, 512 * th:512 * (th + 1)], mm,
                            dq_t[:, fi:fi + 1])
                    qk[fi] = qkt

                # ---- v (token-major with interleaved ones column)
                vaug = []
                for g in range(NG):
                    mm = ps.tile([128, 512], F32, tag="mm", bufs=2,
                                 name=f"vp_{img}_{g}")
                    for c4 in range(4):
                        nc.tensor.matmul(
                            mm, zc[c4][:, 128 * g:128 * (g + 1)],
                            wqkv[c4][:, 2 * C:3 * C],
                            start=(c4 == 0), stop=(c4 == 3))
                    va = sb.tile([128, 33 * NH], BF16, tag="vaug", bufs=8,
                                 name=f"va_{img}_{g}")
                    nc.vector.memset(_ap(va, 32, [[33, NH]]), 1.0)
                    nc.vector.tensor_tensor(
                        out=_ap(va, 0, [[33, NH], [1, 32]]),
                        in0=_ap(mm, 0, [[32, NH], [1, 32]]),
                        in1=_ap(dvb, 0, [[32, NH], [1, 32]]),
                        op=ALU.add)
                    vaug.append(va)
                return qk, vaug

            # ---------------- per-image pipeline ----------------
            _xc0, _zc0 = load_ln1(0)
            _qk0, _va0 = qkv_v(_zc0, 0)
            nxt = (_xc0, _qk0, _va0)
            for img in range(BI):
                xc, qk, vaug = nxt

                # ---- attention, per head-quarter
                atc = [sb.tile([128, 512], BF16, tag="atc", bufs=8,
                               name=f"atc_{img}_{g}") for g in range(NG)]
                for qt in range(4):
                    qh = sb.tile([32, 4 * 1024], BF16, tag="qh", bufs=2,
                                 name=f"qh_{img}_{qt}")
                    kh = sb.tile([32, 4 * 1024], BF16, tag="kh", bufs=1,
                                 name=f"kh_{img}_{qt}")
                    for b4 in range(4):
                        nc.sync.dma_start(
                            out=qh[0:32, 1024 * b4:1024 * (b4 + 1)],
                            in_=qk[qt][32 * b4:32 * (b4 + 1), :])
                        nc.sync.dma_start(
                            out=kh[0:32, 1024 * b4:1024 * (b4 + 1)],
                            in_=qk[4 + qt][32 * b4:32 * (b4 + 1), :])
                    for g in range(NG):
                        stp = ps.tile([128, 512], F32, tag="st", bufs=2,
                                      name=f"stp_{img}_{qt}_{g}")
                        for b4 in range(4):
                            sl = slice(1024 * b4 + 128 * g,
                                       1024 * b4 + 128 * (g + 1))
                            nc.tensor.matmul(
                                stp[:, 128 * b4:128 * (b4 + 1)],
                                kh[0:32, sl], qh[0:32, sl],
                                start=True, stop=True)
                        pt = sb.tile([128, 512], BF16, tag="pt", bufs=3,
                                     name=f"pt_{img}_{qt}_{g}")
                        nc.scalar.activation(pt, stp, AF.Exp)
                        nc.vector.tensor_mul(
                            pt, pt, mask_t[:, 512 * qt:512 * (qt + 1)])
                        av = ps.tile([128, 132], F32, tag="mm", bufs=2,
                                     name=f"av_{img}_{qt}_{g}")
                        for b4 in range(4):
                            h = 4 * qt + b4
                            nc.tensor.matmul(
                                av[:, 33 * b4:33 * (b4 + 1)],
                                pt[:, 128 * b4:128 * (b4 + 1)],
                                vaug[g][:, 33 * h:33 * (h + 1)],
                                start=True, stop=True)
                        rec = sb.tile([128, 4], F32, tag="rec", bufs=2,
                                      name=f"rec_{img}_{qt}_{g}")
                        nc.vector.reciprocal(rec, _ap(av, 32, [[33, 4]]))
                        nc.vector.tensor_tensor(
                            out=_ap(atc[g], 128 * qt, [[32, 4], [1, 32]]),
                            in0=_ap(av, 0, [[33, 4], [1, 32]]),
                            in1=_ap(rec, 0, [[1, 4], [0, 32]]),
                            op=ALU.mult)

                # ---- transpose attention output to channel-major
                actn = []
                for fp in range(4):
                    at = sb.tile([128, 1024], BF16, tag="actn", bufs=4,
                                 name=f"actn_{img}_{fp}")
                    for Q in range(2):
                        tp = ps.tile([128, 512], BF16, tag="av", bufs=2,
                                     name=f"tp_{img}_{fp}_{Q}")
                        for gq in range(4):
                            g = 4 * Q + gq
                            nc.tensor.transpose(
                                tp[:, 128 * gq:128 * (gq + 1)],
                                atc[g][:, 128 * fp:128 * (fp + 1)], idm)
                        nc.scalar.copy(at[:, 512 * Q:512 * (Q + 1)], tp)
                    actn.append(at)

                # ---- proj + residual (in-place xh into xc, window->raster)
                for fo in range(4):
                    for th in range(2):
                        mm = ps.tile([128, 512], F32, tag="mm", bufs=2,
                                     name=f"pj_{img}_{fo}_{th}")
                        for c4 in range(4):
                            nc.tensor.matmul(
                                mm, wproj[c4][:, 128 * fo:128 * (fo + 1)],
                                actn[c4][:, 512 * th:512 * (th + 1)],
                                start=(c4 == 0), stop=(c4 == 3))
                        xap = xc[fo][:, 512 * th:512 * (th + 1)]
                        nc.vector.scalar_tensor_tensor(
                            xap, mm, pb_t[:, fo:fo + 1], xap,
                            ALU.add, ALU.add)

                # prefetch next image's LN1 + qkv/v under this image's tail
                if img + 1 < BI:
                    _xcn, _zcn = load_ln1(img + 1)
                    _qkn, _van = qkv_v(_zcn, img + 1)
                    nxt = (_xcn, _qkn, _van)

                r2_bc, mur2_bc = layernorm(xc, f"l2_{img}")
                z2c = z_pass(xc, r2_bc, mur2_bc, f"l2_{img}")

                # ---- MLP
                for th in range(2):
                    gel = []
                    for f16 in range(16):
                        mm = ps.tile([128, 512], F32, tag="mm", bufs=2,
                                     name=f"m1_{img}_{th}_{f16}")
                        for c4 in range(4):
                            nc.tensor.matmul(
                                mm, wm1[c4][:, 128 * f16:128 * (f16 + 1)],
                                z2c[c4][:, 512 * th:512 * (th + 1)],
                                start=(c4 == 0), stop=(c4 == 3))
                        gt = sb.tile([128, 512], BF16, tag="gelu", bufs=16,
                                     name=f"g_{img}_{th}_{f16}")
                        nc.scalar.activation(gt, mm, AF.Gelu,
                                             bias=d1_t[:, f16:f16 + 1])
                        gel.append(gt)
                    for fo in range(4):
                        mm2 = ps.tile([128, 512], F32, tag="av", bufs=2,
                                      name=f"m2_{img}_{th}_{fo}")
                        for c16 in range(16):
                            nc.tensor.matmul(
                                mm2, wm2[c16][:, 128 * fo:128 * (fo + 1)],
                                gel[c16], start=(c16 == 0), stop=(c16 == 15))
                        xap = xc[fo][:, 512 * th:512 * (th + 1)]
                        nc.vector.scalar_tensor_tensor(
                            xap, mm2, b2_t[:, fo:fo + 1], xap,
                            ALU.add, ALU.add)

                # ---- store (permute window->raster, then contiguous DMA)
                for c4 in range(4):
                    xo = sb.tile([128, 1024], F32, tag="xraw", bufs=2,
                                 name=f"xo_{img}_{c4}")
                    nc.vector.tensor_copy(_ap(xo, 0, RAS8),
                                          _ap(xc[c4], 0, WIN8).bitcast(F32))
                    nc.sync.dma_start(
                        out=out_d[img, 128 * c4:128 * (c4 + 1), :, :]
                        .rearrange("c h w -> c (h w)"),
                        in_=xo)

    nc.compile()
    return nc


def _prep_weights(inputs):
    """Host-side weight preprocessing (numpy, ~ms)."""
    g1 = np.asarray(inputs["norm1_w"], np.float32)
    b1 = np.asarray(inputs["norm1_b"], np.float32)
    g2 = np.asarray(inputs["norm2_w"], np.float32)
    b2n = np.asarray(inputs["norm2_b"], np.float32)
    wqkv = np.array(inputs["qkv_w"], np.float32)              # [3C, C]
    bqkv = np.array(inputs["qkv_b"], np.float32)
    scale = HD ** -0.5
    wqkv[:C] *= scale
    bqkv = bqkv.copy()
    bqkv[:C] *= scale
    dqkv = wqkv @ b1 + bqkv                                   # [3C]
    wqkvT = (wqkv * g1[None, :]).T                            # [C, 3C]

    wproj = np.asarray(inputs["proj_w"], np.float32)          # [C, C]
    pb = np.asarray(inputs["proj_b"], np.float32)
    wm1 = np.asarray(inputs["mlp_w1"], np.float32)            # [MLP_H, C]
    d1 = wm1 @ b2n + np.asarray(inputs["mlp_b1"], np.float32)
    wm1T = (wm1 * g2[None, :]).T                              # [C, MLP_H]
    wm2 = np.asarray(inputs["mlp_w2"], np.float32)            # [C, MLP_H]
    b2o = np.asarray(inputs["mlp_b2"], np.float32)

    rpb = np.asarray(inputs["rpb_table"], np.float32)         # [(2ws-1)^2, NH]
    rel = _relative_position_index(WS)                        # [N, N] (n, m)
    bias = rpb[rel.reshape(-1)].reshape(N, N, NH)             # [n, m, h]
    eb = np.exp(bias)
    mask = np.zeros((128, NH, 128), np.float32)
    for wdx in range(8):
        # tile entry [k, h, q]: k = 16w + m, q = 16w + n -> eb[n, m, h]
        mask[16 * wdx:16 * (wdx + 1), :, 16 * wdx:16 * (wdx + 1)] = \
            eb.transpose(1, 2, 0)
    mask2d = np.ascontiguousarray(
        mask.reshape(128, NH * 128))

    return {
        "wqkv": np.ascontiguousarray(wqkvT).astype(BF),
        "dq": np.ascontiguousarray(
            dqkv[:2 * C].reshape(8, 128).T).astype(np.float32),
        "dvrow": dqkv[2 * C:].reshape(1, C).astype(np.float32),
        "wproj": np.ascontiguousarray(wproj.T).astype(BF),
        "pb": np.ascontiguousarray(pb.reshape(4, 128).T).astype(np.float32),
        "wm1": np.ascontiguousarray(wm1T).astype(BF),
        "d1": np.ascontiguousarray(d1.reshape(16, 128).T).astype(np.float32),
        "wm2": np.ascontiguousarray(wm2.T).astype(BF),
        "b2": np.ascontiguousarray(b2o.reshape(4, 128).T).astype(np.float32),
        "mask": mask2d.astype(BF),
        "idm": np.eye(128, dtype=BF),
        "onesc": np.ones((128, 1), np.float32),
    }


def get_program():
    if "nc" not in _cache:
        _cache["nc"] = _build_program()
    return _cache["nc"]


def make_in_maps(inputs):
    wmaps = _prep_weights(inputs)
    x_full = np.asarray(inputs["x"], np.float32)
    in_maps = []
    for core in range(NCORES):
        m = dict(wmaps)
        m["x"] = np.ascontiguousarray(x_full[BI * core:BI * (core + 1)])
        in_maps.append(m)
    return in_maps


def kernel(**inputs):
    nc = get_program()
    in_maps = make_in_maps(inputs)
    res = run_bass_kernel_spmd(nc, in_maps, list(range(NCORES)))
    out = np.concatenate([res.results[c]["out"] for c in range(NCORES)],
                         axis=0)
    return out


# revision 38
# speedup vs baseline: 5959.5244x; 1.0242x over previous
"""Swin-style windowed-attention block (LN->W-MSA->residual->LN->MLP->residual)
for TRN2, data-parallel over batch across 8 NeuronCores.

Layout strategy: channels-on-partitions (CT) end to end; x arrives [B,C,H,W]
which is already channel-major per image. LayerNorm stats via ones-matmuls on
the PE (partition-dim sums), per-token scale/shift via DRAM-round-trip
partition broadcasts. Attention computes S-transposed (keys on partitions)
from per-head partition-0 tiles produced by an SBUF->SBUF DMA rearrangement;
softmax without max-subtraction (scores are small by construction); the
relative-position bias and the block-diagonal window mask are folded into one
host-precomputed multiplicative exp(bias) mask; row-sums ride along the AV
matmul as a ones-column of the value matrix; AV uses P~ as the stationary
operand giving token-major outputs that a cheap PE transpose returns to CT.
"""
import sys
import numpy as np
import ml_dtypes

sys.path.insert(0, "/opt/trn_rl_repo")

import concourse.bass as bass
import concourse.bacc as bacc
import concourse.tile as tile
from concourse import mybir
from concourse.bass_utils import run_bass_kernel_spmd

F32 = mybir.dt.float32
F32R = mybir.dt.float32r
BF16 = mybir.dt.bfloat16
AF = mybir.ActivationFunctionType
ALU = mybir.AluOpType
BF = ml_dtypes.bfloat16

# problem constants (hardcoded per the task contract)
B, C, H, W = 32, 512, 32, 32
NH, WS = 16, 4
HD = C // NH            # 32
N = WS * WS             # 16 tokens per window
EPS = 1e-5
MLP_H = 4 * C           # 2048
NCORES = 8
BI = B // NCORES        # images per core = 4
T = H * W               # tokens per image = 1024
NG = T // 128           # 128-token groups per image = 8

_cache = {}


def _relative_position_index(ws):
    coords = np.stack(np.meshgrid(np.arange(ws), np.arange(ws), indexing="ij"))
    cf = coords.reshape(2, -1)
    rel = cf[:, :, None] - cf[:, None, :]
    rel = rel.transpose(1, 2, 0).astype(np.int64)
    rel[:, :, 0] += ws - 1
    rel[:, :, 1] += ws - 1
    rel[:, :, 0] *= 2 * ws - 1
    return rel.sum(-1)


def _ap(t, off, dims):
    return bass.AP(tensor=t.tensor, offset=t.offset + off,
                   ap=[t.ap[0]] + [list(d) for d in dims])

# window permutation: window-ordered col = 128g + 16ww + 4i + j
#                     raster col         = 128g + 32i + 4ww + j
WIN4 = [[128, 4], [16, 8], [4, 4], [1, 4]]    # half-image (4 groups)
RAS4 = [[128, 4], [4, 8], [32, 4], [1, 4]]
WIN8 = [[128, 8], [16, 8], [4, 4], [1, 4]]    # full image (8 groups)
RAS8 = [[128, 8], [4, 8], [32, 4], [1, 4]]


def _build_program():
    nc = bacc.Bacc("TRN2", target_bir_lowering=False, debug=False,
                   enable_asserts=True, num_devices=NCORES)

    def din(name, shape, dt):
        return nc.dram_tensor(name, shape, dt, kind="ExternalInput").ap()

    x_d = din("x", (BI, C, H, W), F32)
    wqkv_d = din("wqkv", (C, 3 * C), BF16)        # [c, f] = diag(g1) @ Wqkv.T
    dq_d = din("dq", (128, 8), F32)               # q/k bias, col per f-tile
    dv_d = din("dvrow", (1, C), F32)              # v bias row (bcast source)
    wproj_d = din("wproj", (C, C), BF16)
    pb_d = din("pb", (128, 4), F32)
    wm1_d = din("wm1", (C, MLP_H), BF16)
    d1_d = din("d1", (128, 16), F32)
    wm2_d = din("wm2", (MLP_H, C), BF16)
    b2_d = din("b2", (128, 4), F32)
    mask_d = din("mask", (128, NH * 128), BF16)
    idm_d = din("idm", (128, 128), BF16)
    ones_d = din("onesc", (128, 1), F32)

    out_d = nc.dram_tensor("out", (BI, C, H, W), F32, kind="ExternalOutput").ap()

    with tile.TileContext(nc) as tc:
        with tc.tile_pool(name="sb", bufs=1) as sb, \
             tc.tile_pool(name="dr", bufs=1, space="DRAM") as dr, \
             tc.tile_pool(name="ps", bufs=1, space="PSUM") as ps:

            # ---------------- resident constants ----------------
            wqkv = []
            for c4 in range(4):
                wq_t = sb.tile([128, 3 * C], BF16, tag="wqkv", bufs=4,
                               name=f"wqkv{c4}")
                nc.sync.dma_start(out=wq_t, in_=wqkv_d[128 * c4:128 * (c4 + 1), :])
                wqkv.append(wq_t)
            wproj = []
            for c4 in range(4):
                wp_t = sb.tile([128, C], BF16, tag="wproj", bufs=4,
                               name=f"wproj{c4}")
                nc.sync.dma_start(out=wp_t, in_=wproj_d[128 * c4:128 * (c4 + 1), :])
                wproj.append(wp_t)
            wm1 = []
            for c4 in range(4):
                w1_t = sb.tile([128, MLP_H], BF16, tag="wm1", bufs=4,
                               name=f"wm1{c4}")
                nc.sync.dma_start(out=w1_t, in_=wm1_d[128 * c4:128 * (c4 + 1), :])
                wm1.append(w1_t)
            wm2 = []
            for c16 in range(16):
                w2_t = sb.tile([128, C], BF16, tag="wm2", bufs=16,
                               name=f"wm2{c16}")
                nc.sync.dma_start(out=w2_t, in_=wm2_d[128 * c16:128 * (c16 + 1), :])
                wm2.append(w2_t)
            mask_t = sb.tile([128, NH * 128], BF16, tag="mask", name="mask_t")
            nc.sync.dma_start(out=mask_t, in_=mask_d)
            idm = sb.tile([128, 128], BF16, tag="idm", name="idm")
            nc.sync.dma_start(out=idm, in_=idm_d)
            onesr = sb.tile([128, 1], F32R, tag="onesr", name="onesr")
            nc.sync.dma_start(out=onesr, in_=ones_d.bitcast(F32R))
            onesb = sb.tile([128, 1], BF16, tag="onesb", name="onesb")
            nc.vector.memset(onesb, 1.0)
            dq_t = sb.tile([128, 8], F32, tag="dq", name="dq_t")
            nc.sync.dma_start(out=dq_t, in_=dq_d)
            pb_t = sb.tile([128, 4], F32, tag="pbt", name="pb_t")
            nc.sync.dma_start(out=pb_t, in_=pb_d)
            d1_t = sb.tile([128, 16], F32, tag="d1t", name="d1_t")
            nc.sync.dma_start(out=d1_t, in_=d1_d)
            b2_t = sb.tile([128, 4], F32, tag="b2t", name="b2_t")
            nc.sync.dma_start(out=b2_t, in_=b2_d)
            dvb = sb.tile([128, C], F32, tag="dvb", name="dvb")
            nc.sync.dma_start(out=dvb, in_=bass.AP(
                tensor=dv_d.tensor, offset=dv_d.offset, ap=[[0, 128], [1, C]]))
            eps_t = sb.tile([1, 1], F32, tag="eps", name="eps_t")
            nc.vector.memset(eps_t, EPS)

            def layernorm(xc, sfx):
                """xc: 4 chunk tiles [128,1024] F32R -> (r_bc, mur_bc)."""
                st = sb.tile([128, 1024], F32, tag="stat", bufs=2,
                             name=f"st_{sfx}")
                for hh in range(2):
                    sl = slice(512 * hh, 512 * (hh + 1))
                    sx = ps.tile([128, 512], F32, tag="sx", bufs=1,
                                 name=f"sx_{sfx}_{hh}")
                    for c4 in range(4):
                        nc.tensor.matmul(sx[0:1, :], onesr, xc[c4][:, sl],
                                         start=(c4 == 0), stop=(c4 == 3))
                    sx2 = ps.tile([128, 512], F32, tag="sx2", bufs=1,
                                  name=f"sx2_{sfx}_{hh}")
                    for c4 in range(4):
                        x2 = sb.tile([128, 512], BF16, tag="zt", bufs=2,
                                     name=f"x2_{sfx}_{hh}_{c4}")
                        nc.vector.tensor_mul(x2, xc[c4][:, sl].bitcast(F32),
                                             xc[c4][:, sl].bitcast(F32))
                        nc.tensor.matmul(sx2[0:1, :], onesb, x2,
                                         start=(c4 == 0), stop=(c4 == 3))
                    # mu2 = (sum(x)/C)^2 straight from psum
                    nc.scalar.activation(st[0:1, sl], sx[0:1, :], AF.Square,
                                         scale=1.0 / C)
                    nc.vector.scalar_tensor_tensor(st[64:65, sl], sx2[0:1, :],
                                                   1.0 / C, st[0:1, sl],
                                                   ALU.mult, ALU.subtract)
                    nc.scalar.activation(st[96:97, sl], st[64:65, sl], AF.Ln,
                                         bias=eps_t)
                    nc.scalar.activation(st[32:33, sl], st[96:97, sl], AF.Exp,
                                         scale=-0.5)
                    # mur = (sum(x)/C) * r ; psum in0 is exempt from the
                    # same-base-partition constraint
                    nc.vector.scalar_tensor_tensor(st[64:65, sl], sx[0:1, :],
                                                   1.0 / C, st[32:33, sl],
                                                   ALU.mult, ALU.mult)
                row = dr.tile([2, 1024], BF16, tag="rt", bufs=2,
                              name=f"row_{sfx}")
                # casting DMAs (gpsimd) write r and mur rows
                nc.gpsimd.dma_start(out=row[0:1, :], in_=st[32:33, :])
                nc.gpsimd.dma_start(out=row[1:2, :], in_=st[64:65, :])
                rm_bc = sb.tile([128, 2048], BF16, tag="bc", bufs=1,
                                name=f"rmbc_{sfx}")
                nc.sync.dma_start(out=rm_bc, in_=bass.AP(
                    tensor=row.tensor, offset=row.offset,
                    ap=[[0, 128], [1, 2048]]))
                return rm_bc[:, 0:1024], rm_bc[:, 1024:2048]

            def z_pass(xc, r_bc, mur_bc, sfx):
                zc = []
                for c4 in range(4):
                    t1 = sb.tile([128, 1024], BF16, tag="zt", bufs=2,
                                 name=f"t1_{sfx}_{c4}")
                    nc.vector.tensor_mul(t1, xc[c4].bitcast(F32), r_bc)
                    z = sb.tile([128, 1024], BF16, tag="z", bufs=8,
                                name=f"z_{sfx}_{c4}")
                    nc.vector.tensor_tensor(out=z, in0=t1, in1=mur_bc,
                                            op=ALU.subtract)
                    zc.append(z)
                return zc

            def load_ln1(img):
                xc = []
                for c4 in range(4):
                    xraw = sb.tile([128, 1024], F32, tag="xraw", bufs=2,
                                   name=f"xr_{img}_{c4}")
                    nc.sync.dma_start(
                        out=xraw,
                        in_=x_d[img, 128 * c4:128 * (c4 + 1), :, :]
                        .rearrange("c h w -> c (h w)"))
                    xt = sb.tile([128, 1024], F32R, tag="xc", bufs=8,
                                 name=f"x_{img}_{c4}")
                    nc.vector.tensor_copy(_ap(xt, 0, WIN8),
                                          _ap(xraw, 0, RAS8).bitcast(F32R))
                    xc.append(xt)
                with tc.high_priority():
                    r_bc, mur_bc = layernorm(xc, f"l1_{img}")
                zc = z_pass(xc, r_bc, mur_bc, f"l1_{img}")
                return xc, zc


            def qkv_v(zc, img):
                # ---- qkv q/k f-tiles
                qk = {}
                for fi in (0, 4, 1, 5, 2, 6, 3, 7):
                    qkt = sb.tile([128, 1024], BF16, tag="qk", bufs=7,
                                  name=f"qk_{img}_{fi}")
                    for th in range(2):
                        mm = ps.tile([128, 512], F32, tag="mm", bufs=2,
                                     name=f"qkp_{img}_{fi}_{th}")
                        for c4 in range(4):
                            nc.tensor.matmul(
                                mm, wqkv[c4][:, 128 * fi:128 * (fi + 1)],
                                zc[c4][:, 512 * th:512 * (th + 1)],
                                start=(c4 == 0), stop=(c4 == 3))
                        with tc.high_priority():
                            nc.vector.tensor_scalar_add(
                                qkt[:, 512 * th:512 * (th + 1)], mm,
                                dq_t[:, fi:fi + 1])
                    qk[fi] = qkt

                # ---- v (token-major with interleaved ones column)
                vaug = []
                for g in range(NG):
                    mm = ps.tile([128, 512], F32, tag="mm", bufs=2,
                                 name=f"vp_{img}_{g}")
                    for c4 in range(4):
                        nc.tensor.matmul(
                            mm, zc[c4][:, 128 * g:128 * (g + 1)],
                            wqkv[c4][:, 2 * C:3 * C],
                            start=(c4 == 0), stop=(c4 == 3))
                    va = sb.tile([128, 33 * NH], BF16, tag="vaug", bufs=8,
                                 name=f"va_{img}_{g}")
                    nc.vector.memset(_ap(va, 32, [[33, NH]]), 1.0)
                    nc.vector.tensor_tensor(
                        out=_ap(va, 0, [[33, NH], [1, 32]]),
                        in0=_ap(mm, 0, [[32, NH], [1, 32]]),
                        in1=_ap(dvb, 0, [[32, NH], [1, 32]]),
                        op=ALU.add)
                    vaug.append(va)
                return qk, vaug

            # ---------------- per-image pipeline ----------------
            _xc0, _zc0 = load_ln1(0)
            _qk0, _va0 = qkv_v(_zc0, 0)
            nxt = (_xc0, _qk0, _va0)
            for img in range(BI):
                xc, qk, vaug = nxt

                # ---- attention, per head-quarter
                atc = [sb.tile([128, 512], BF16, tag="atc", bufs=8,
                               name=f"atc_{img}_{g}") for g in range(NG)]
                for qt in range(4):
                    qh = sb.tile([32, 4 * 1024], BF16, tag="qh", bufs=2,
                                 name=f"qh_{img}_{qt}")
                    kh = sb.tile([32, 4 * 1024], BF16, tag="kh", bufs=1,
                                 name=f"kh_{img}_{qt}")
                    for b4 in range(4):
                        nc.sync.dma_start(
                            out=qh[0:32, 1024 * b4:1024 * (b4 + 1)],
                            in_=qk[qt][32 * b4:32 * (b4 + 1), :])
                        nc.sync.dma_start(
                            out=kh[0:32, 1024 * b4:1024 * (b4 + 1)],
                            in_=qk[4 + qt][32 * b4:32 * (b4 + 1), :])
                    for g in range(NG):
                        stp = ps.tile([128, 512], F32, tag="st", bufs=2,
                                      name=f"stp_{img}_{qt}_{g}")
                        for b4 in range(4):
                            sl = slice(1024 * b4 + 128 * g,
                                       1024 * b4 + 128 * (g + 1))
                            nc.tensor.matmul(
                                stp[:, 128 * b4:128 * (b4 + 1)],
                                kh[0:32, sl], qh[0:32, sl],
                                start=True, stop=True)
                        pt = sb.tile([128, 512], BF16, tag="pt", bufs=3,
                                     name=f"pt_{img}_{qt}_{g}")
                        nc.scalar.activation(pt, stp, AF.Exp)
                        nc.vector.tensor_mul(
                            pt, pt, mask_t[:, 512 * qt:512 * (qt + 1)])
                        av = ps.tile([128, 132], F32, tag="mm", bufs=2,
                                     name=f"av_{img}_{qt}_{g}")
                        for b4 in range(4):
                            h = 4 * qt + b4
                            nc.tensor.matmul(
                                av[:, 33 * b4:33 * (b4 + 1)],
                                pt[:, 128 * b4:128 * (b4 + 1)],
                                vaug[g][:, 33 * h:33 * (h + 1)],
                                start=True, stop=True)
                        rec = sb.tile([128, 4], F32, tag="rec", bufs=2,
                                      name=f"rec_{img}_{qt}_{g}")
                        nc.vector.reciprocal(rec, _ap(av, 32, [[33, 4]]))
                        nc.vector.tensor_tensor(
                            out=_ap(atc[g], 128 * qt, [[32, 4], [1, 32]]),
                            in0=_ap(av, 0, [[33, 4], [1, 32]]),
                            in1=_ap(rec, 0, [[1, 4], [0, 32]]),
                            op=ALU.mult)

                # ---- transpose attention output to channel-major
                actn = []
                for fp in range(4):
                    at = sb.tile([128, 1024], BF16, tag="actn", bufs=4,
                                 name=f"actn_{img}_{fp}")
                    for Q in range(2):
                        tp = ps.tile([128, 512], BF16, tag="av", bufs=2,
                                     name=f"tp_{img}_{fp}_{Q}")
                        for gq in range(4):
                            g = 4 * Q + gq
                            nc.tensor.transpose(
                                tp[:, 128 * gq:128 * (gq + 1)],
                                atc[g][:, 128 * fp:128 * (fp + 1)], idm)
                        nc.scalar.copy(at[:, 512 * Q:512 * (Q + 1)], tp)
                    actn.append(at)

                # ---- proj + residual (in-place xh into xc, window->raster)
                for fo in range(4):
                    for th in range(2):
                        mm = ps.tile([128, 512], F32, tag="mm", bufs=2,
                                     name=f"pj_{img}_{fo}_{th}")
                        for c4 in range(4):
                            nc.tensor.matmul(
                                mm, wproj[c4][:, 128 * fo:128 * (fo + 1)],
                                actn[c4][:, 512 * th:512 * (th + 1)],
                                start=(c4 == 0), stop=(c4 == 3))
                        xap = xc[fo][:, 512 * th:512 * (th + 1)]
                        nc.vector.scalar_tensor_tensor(
                            xap, mm, pb_t[:, fo:fo + 1], xap,
                            ALU.add, ALU.add)

                # prefetch next image's LN1 + qkv/v under this image's tail
                if img + 1 < BI:
                    _xcn, _zcn = load_ln1(img + 1)
                    _qkn, _van = qkv_v(_zcn, img + 1)
                    nxt = (_xcn, _qkn, _van)

                with tc.high_priority():
                    r2_bc, mur2_bc = layernorm(xc, f"l2_{img}")
                z2c = z_pass(xc, r2_bc, mur2_bc, f"l2_{img}")

                # ---- MLP
                for th in range(2):
                    gel = []
                    for f16 in range(16):
                        mm = ps.tile([128, 512], F32, tag="mm", bufs=2,
                                     name=f"m1_{img}_{th}_{f16}")
                        for c4 in range(4):
                            nc.tensor.matmul(
                                mm, wm1[c4][:, 128 * f16:128 * (f16 + 1)],
                                z2c[c4][:, 512 * th:512 * (th + 1)],
                                start=(c4 == 0), stop=(c4 == 3))
                        gt = sb.tile([128, 512], BF16, tag="gelu", bufs=16,
                                     name=f"g_{img}_{th}_{f16}")
                        nc.scalar.activation(gt, mm, AF.Gelu,
                                             bias=d1_t[:, f16:f16 + 1])
                        gel.append(gt)
                    for fo in range(4):
                        mm2 = ps.tile([128, 512], F32, tag="av", bufs=2,
                                      name=f"m2_{img}_{th}_{fo}")
                        for c16 in range(16):
                            nc.tensor.matmul(
                                mm2, wm2[c16][:, 128 * fo:128 * (fo + 1)],
                                gel[c16], start=(c16 == 0), stop=(c16 == 15))
                        xap = xc[fo][:, 512 * th:512 * (th + 1)]
                        nc.vector.scalar_tensor_tensor(
                            xap, mm2, b2_t[:, fo:fo + 1], xap,
                            ALU.add, ALU.add)

                # ---- store (permute window->raster, then contiguous DMA)
                for c4 in range(4):
                    xo = sb.tile([128, 1024], F32, tag="xraw", bufs=2,
                                 name=f"xo_{img}_{c4}")
                    nc.vector.tensor_copy(_ap(xo, 0, RAS8),
                                          _ap(xc[c4], 0, WIN8).bitcast(F32))
                    nc.sync.dma_start(
                        out=out_d[img, 128 * c4:128 * (c4 + 1), :, :]
                        .rearrange("c h w -> c (h w)"),
                        in_=xo)

    nc.compile()
    return nc


def _prep_weights(inputs):
    """Host-side weight preprocessing (numpy, ~ms)."""
    g1 = np.asarray(inputs["norm1_w"], np.float32)
    b1 = np.asarray(inputs["norm1_b"], np.float32)
    g2 = np.asarray(inputs["norm2_w"], np.float32)
    b2n = np.asarray(inputs["norm2_b"], np.float32)
    wqkv = np.array(inputs["qkv_w"], np.float32)              # [3C, C]
    bqkv = np.array(inputs["qkv_b"], np.float32)
    scale = HD ** -0.5
    wqkv[:C] *= scale
    bqkv = bqkv.copy()
    bqkv[:C] *= scale
    dqkv = wqkv @ b1 + bqkv                                   # [3C]
    wqkvT = (wqkv * g1[None, :]).T                            # [C, 3C]

    wproj = np.asarray(inputs["proj_w"], np.float32)          # [C, C]
    pb = np.asarray(inputs["proj_b"], np.float32)
    wm1 = np.asarray(inputs["mlp_w1"], np.float32)            # [MLP_H, C]
    d1 = wm1 @ b2n + np.asarray(inputs["mlp_b1"], np.float32)
    wm1T = (wm1 * g2[None, :]).T                              # [C, MLP_H]
    wm2 = np.asarray(inputs["mlp_w2"], np.float32)            # [C, MLP_H]
    b2o = np.asarray(inputs["mlp_b2"], np.float32)

    rpb = np.asarray(inputs["rpb_table"], np.float32)         # [(2ws-1)^2, NH]
    rel = _relative_position_index(WS)                        # [N, N] (n, m)
    bias = rpb[rel.reshape(-1)].reshape(N, N, NH)             # [n, m, h]
    eb = np.exp(bias)
    mask = np.zeros((128, NH, 128), np.float32)
    for wdx in range(8):
        # tile entry [k, h, q]: k = 16w + m, q = 16w + n -> eb[n, m, h]
        mask[16 * wdx:16 * (wdx + 1), :, 16 * wdx:16 * (wdx + 1)] = \
            eb.transpose(1, 2, 0)
    mask2d = np.ascontiguousarray(
        mask.reshape(128, NH * 128))

    return {
        "wqkv": np.ascontiguousarray(wqkvT).astype(BF),
        "dq": np.ascontiguousarray(
            dqkv[:2 * C].reshape(8, 128).T).astype(np.float32),
        "dvrow": dqkv[2 * C:].reshape(1, C).astype(np.float32),
        "wproj": np.ascontiguousarray(wproj.T).astype(BF),
        "pb": np.ascontiguousarray(pb.reshape(4, 128).T).astype(np.float32),
        "wm1": np.ascontiguousarray(wm1T).astype(BF),
        "d1": np.ascontiguousarray(d1.reshape(16, 128).T).astype(np.float32),
        "wm2": np.ascontiguousarray(wm2.T).astype(BF),
        "b2": np.ascontiguousarray(b2o.reshape(4, 128).T).astype(np.float32),
        "mask": mask2d.astype(BF),
        "idm": np.eye(128, dtype=BF),
        "onesc": np.ones((128, 1), np.float32),
    }


def get_program():
    if "nc" not in _cache:
        _cache["nc"] = _build_program()
    return _cache["nc"]


def make_in_maps(inputs):
    wmaps = _prep_weights(inputs)
    x_full = np.asarray(inputs["x"], np.float32)
    in_maps = []
    for core in range(NCORES):
        m = dict(wmaps)
        m["x"] = np.ascontiguousarray(x_full[BI * core:BI * (core + 1)])
        in_maps.append(m)
    return in_maps


def kernel(**inputs):
    nc = get_program()
    in_maps = make_in_maps(inputs)
    res = run_bass_kernel_spmd(nc, in_maps, list(range(NCORES)))
    out = np.concatenate([res.results[c]["out"] for c in range(NCORES)],
                         axis=0)
    return out


# revision 43
# speedup vs baseline: 5969.5137x; 1.0017x over previous
"""Swin-style windowed-attention block (LN->W-MSA->residual->LN->MLP->residual)
for TRN2, data-parallel over batch across 8 NeuronCores.

Layout strategy: channels-on-partitions (CT) end to end; x arrives [B,C,H,W]
which is already channel-major per image. LayerNorm stats via ones-matmuls on
the PE (partition-dim sums), per-token scale/shift via DRAM-round-trip
partition broadcasts. Attention computes S-transposed (keys on partitions)
from per-head partition-0 tiles produced by an SBUF->SBUF DMA rearrangement;
softmax without max-subtraction (scores are small by construction); the
relative-position bias and the block-diagonal window mask are folded into one
host-precomputed multiplicative exp(bias) mask; row-sums ride along the AV
matmul as a ones-column of the value matrix; AV uses P~ as the stationary
operand giving token-major outputs that a cheap PE transpose returns to CT.
"""
import sys
import numpy as np
import ml_dtypes

sys.path.insert(0, "/opt/trn_rl_repo")

import concourse.bass as bass
import concourse.bacc as bacc
import concourse.tile as tile
from concourse import mybir
from concourse.bass_utils import run_bass_kernel_spmd

F32 = mybir.dt.float32
F32R = mybir.dt.float32r
BF16 = mybir.dt.bfloat16
AF = mybir.ActivationFunctionType
ALU = mybir.AluOpType
BF = ml_dtypes.bfloat16

# problem constants (hardcoded per the task contract)
B, C, H, W = 32, 512, 32, 32
NH, WS = 16, 4
HD = C // NH            # 32
N = WS * WS             # 16 tokens per window
EPS = 1e-5
MLP_H = 4 * C           # 2048
NCORES = 8
BI = B // NCORES        # images per core = 4
T = H * W               # tokens per image = 1024
NG = T // 128           # 128-token groups per image = 8

_cache = {}


def _relative_position_index(ws):
    coords = np.stack(np.meshgrid(np.arange(ws), np.arange(ws), indexing="ij"))
    cf = coords.reshape(2, -1)
    rel = cf[:, :, None] - cf[:, None, :]
    rel = rel.transpose(1, 2, 0).astype(np.int64)
    rel[:, :, 0] += ws - 1
    rel[:, :, 1] += ws - 1
    rel[:, :, 0] *= 2 * ws - 1
    return rel.sum(-1)


def _ap(t, off, dims):
    return bass.AP(tensor=t.tensor, offset=t.offset + off,
                   ap=[t.ap[0]] + [list(d) for d in dims])

# window permutation: window-ordered col = 128g + 16ww + 4i + j
#                     raster col         = 128g + 32i + 4ww + j
WIN4 = [[128, 4], [16, 8], [4, 4], [1, 4]]    # half-image (4 groups)
RAS4 = [[128, 4], [4, 8], [32, 4], [1, 4]]
WIN8 = [[128, 8], [16, 8], [4, 4], [1, 4]]    # full image (8 groups)
RAS8 = [[128, 8], [4, 8], [32, 4], [1, 4]]


def _build_program():
    nc = bacc.Bacc("TRN2", target_bir_lowering=False, debug=False,
                   enable_asserts=True, num_devices=NCORES)

    def din(name, shape, dt):
        return nc.dram_tensor(name, shape, dt, kind="ExternalInput").ap()

    x_d = din("x", (BI, C, H, W), F32)
    wqkv_d = din("wqkv", (C, 3 * C), BF16)        # [c, f] = diag(g1) @ Wqkv.T
    dq_d = din("dq", (128, 8), F32)               # q/k bias, col per f-tile
    dv_d = din("dvrow", (1, C), F32)              # v bias row (bcast source)
    wproj_d = din("wproj", (C, C), BF16)
    pb_d = din("pb", (128, 4), F32)
    wm1_d = din("wm1", (C, MLP_H), BF16)
    d1_d = din("d1", (128, 16), F32)
    wm2_d = din("wm2", (MLP_H, C), BF16)
    b2_d = din("b2", (128, 4), F32)
    mask_d = din("mask", (128, NH * 128), BF16)
    idm_d = din("idm", (128, 128), BF16)
    ones_d = din("onesc", (128, 1), F32)

    out_d = nc.dram_tensor("out", (BI, C, H, W), F32, kind="ExternalOutput").ap()

    with tile.TileContext(nc) as tc:
        with tc.tile_pool(name="sb", bufs=1) as sb, \
             tc.tile_pool(name="dr", bufs=1, space="DRAM") as dr, \
             tc.tile_pool(name="ps", bufs=1, space="PSUM") as ps:

            # ---------------- resident constants ----------------
            wqkv = []
            for c4 in range(4):
                wq_t = sb.tile([128, 3 * C], BF16, tag="wqkv", bufs=4,
                               name=f"wqkv{c4}")
                nc.sync.dma_start(out=wq_t, in_=wqkv_d[128 * c4:128 * (c4 + 1), :])
                wqkv.append(wq_t)
            wproj = []
            for c4 in range(4):
                wp_t = sb.tile([128, C], BF16, tag="wproj", bufs=4,
                               name=f"wproj{c4}")
                nc.sync.dma_start(out=wp_t, in_=wproj_d[128 * c4:128 * (c4 + 1), :])
                wproj.append(wp_t)
            wm1 = []
            for c4 in range(4):
                w1_t = sb.tile([128, MLP_H], BF16, tag="wm1", bufs=4,
                               name=f"wm1{c4}")
                nc.sync.dma_start(out=w1_t, in_=wm1_d[128 * c4:128 * (c4 + 1), :])
                wm1.append(w1_t)
            wm2 = []
            for c16 in range(16):
                w2_t = sb.tile([128, C], BF16, tag="wm2", bufs=16,
                               name=f"wm2{c16}")
                nc.sync.dma_start(out=w2_t, in_=wm2_d[128 * c16:128 * (c16 + 1), :])
                wm2.append(w2_t)
            mask_t = sb.tile([128, NH * 128], BF16, tag="mask", name="mask_t")
            nc.sync.dma_start(out=mask_t, in_=mask_d)
            idm = sb.tile([128, 128], BF16, tag="idm", name="idm")
            nc.sync.dma_start(out=idm, in_=idm_d)
            onesr = sb.tile([128, 1], F32R, tag="onesr", name="onesr")
            nc.sync.dma_start(out=onesr, in_=ones_d.bitcast(F32R))
            onesb = sb.tile([128, 1], BF16, tag="onesb", name="onesb")
            nc.vector.memset(onesb, 1.0)
            dq_t = sb.tile([128, 8], F32, tag="dq", name="dq_t")
            nc.sync.dma_start(out=dq_t, in_=dq_d)
            pb_t = sb.tile([128, 4], F32, tag="pbt", name="pb_t")
            nc.sync.dma_start(out=pb_t, in_=pb_d)
            d1_t = sb.tile([128, 16], F32, tag="d1t", name="d1_t")
            nc.sync.dma_start(out=d1_t, in_=d1_d)
            b2_t = sb.tile([128, 4], F32, tag="b2t", name="b2_t")
            nc.sync.dma_start(out=b2_t, in_=b2_d)
            dvb = sb.tile([128, C], F32, tag="dvb", name="dvb")
            nc.sync.dma_start(out=dvb, in_=bass.AP(
                tensor=dv_d.tensor, offset=dv_d.offset, ap=[[0, 128], [1, C]]))
            eps_t = sb.tile([1, 1], F32, tag="eps", name="eps_t")
            nc.vector.memset(eps_t, EPS)

            def layernorm(xc, sfx):
                """xc: 4 chunk tiles [128,1024] F32R -> (r_bc, mur_bc)."""
                st = sb.tile([128, 1024], F32, tag="stat", bufs=2,
                             name=f"st_{sfx}")
                for hh in range(2):
                    sl = slice(512 * hh, 512 * (hh + 1))
                    sx = ps.tile([128, 512], F32, tag="sx", bufs=1,
                                 name=f"sx_{sfx}_{hh}")
                    for c4 in range(4):
                        nc.tensor.matmul(sx[0:1, :], onesr, xc[c4][:, sl],
                                         start=(c4 == 0), stop=(c4 == 3))
                    sx2 = ps.tile([128, 512], F32, tag="sx2", bufs=1,
                                  name=f"sx2_{sfx}_{hh}")
                    for c4 in range(4):
                        x2 = sb.tile([128, 512], BF16, tag="zt", bufs=2,
                                     name=f"x2_{sfx}_{hh}_{c4}")
                        nc.vector.tensor_mul(x2, xc[c4][:, sl].bitcast(F32),
                                             xc[c4][:, sl].bitcast(F32))
                        nc.tensor.matmul(sx2[0:1, :], onesb, x2,
                                         start=(c4 == 0), stop=(c4 == 3))
                    # mu2 = (sum(x)/C)^2 straight from psum
                    nc.scalar.activation(st[0:1, sl], sx[0:1, :], AF.Square,
                                         scale=1.0 / C)
                    nc.vector.scalar_tensor_tensor(st[64:65, sl], sx2[0:1, :],
                                                   1.0 / C, st[0:1, sl],
                                                   ALU.mult, ALU.subtract)
                    nc.scalar.activation(st[96:97, sl], st[64:65, sl], AF.Ln,
                                         bias=eps_t)
                    nc.scalar.activation(st[32:33, sl], st[96:97, sl], AF.Exp,
                                         scale=-0.5)
                    # mur = (sum(x)/C) * r ; psum in0 is exempt from the
                    # same-base-partition constraint
                    nc.vector.scalar_tensor_tensor(st[64:65, sl], sx[0:1, :],
                                                   1.0 / C, st[32:33, sl],
                                                   ALU.mult, ALU.mult)
                row = dr.tile([2, 1024], BF16, tag="rt", bufs=2,
                              name=f"row_{sfx}")
                # casting DMAs (gpsimd) write r and mur rows
                nc.gpsimd.dma_start(out=row[0:1, :], in_=st[32:33, :])
                nc.gpsimd.dma_start(out=row[1:2, :], in_=st[64:65, :])
                rm_bc = sb.tile([128, 2048], BF16, tag="bc", bufs=1,
                                name=f"rmbc_{sfx}")
                nc.sync.dma_start(out=rm_bc, in_=bass.AP(
                    tensor=row.tensor, offset=row.offset,
                    ap=[[0, 128], [1, 2048]]))
                return rm_bc[:, 0:1024], rm_bc[:, 1024:2048]

            def z_pass(xc, r_bc, mur_bc, sfx):
                zc = []
                for c4 in range(4):
                    t1 = sb.tile([128, 1024], BF16, tag="zt", bufs=2,
                                 name=f"t1_{sfx}_{c4}")
                    z = sb.tile([128, 1024], BF16, tag="z", bufs=8,
                                name=f"z_{sfx}_{c4}")
                    nc.vector.tensor_mul(t1, xc[c4].bitcast(F32), r_bc)
                    nc.vector.tensor_tensor(out=z, in0=t1, in1=mur_bc,
                                            op=ALU.subtract)
                    zc.append(z)
                return zc

            def load_ln1(img):
                xc = []
                for c4 in range(4):
                    xraw = sb.tile([128, 1024], F32, tag="xraw", bufs=2,
                                   name=f"xr_{img}_{c4}")
                    nc.sync.dma_start(
                        out=xraw,
                        in_=x_d[img, 128 * c4:128 * (c4 + 1), :, :]
                        .rearrange("c h w -> c (h w)"))
                    xt = sb.tile([128, 1024], F32R, tag="xc", bufs=8,
                                 name=f"x_{img}_{c4}")
                    nc.vector.tensor_copy(_ap(xt, 0, WIN8),
                                          _ap(xraw, 0, RAS8).bitcast(F32R))
                    xc.append(xt)
                with tc.high_priority():
                    r_bc, mur_bc = layernorm(xc, f"l1_{img}")
                zc = z_pass(xc, r_bc, mur_bc, f"l1_{img}")
                return xc, zc


            def qkv_v(zc, img):
                # ---- qkv q/k f-tiles
                qk = {}
                for fi in (0, 4, 1, 5, 2, 6, 3, 7):
                    qkt = sb.tile([128, 1024], BF16, tag="qk", bufs=7,
                                  name=f"qk_{img}_{fi}")
                    for th in range(2):
                        mm = ps.tile([128, 512], F32, tag="mm", bufs=2,
                                     name=f"qkp_{img}_{fi}_{th}")
                        for c4 in range(4):
                            nc.tensor.matmul(
                                mm, wqkv[c4][:, 128 * fi:128 * (fi + 1)],
                                zc[c4][:, 512 * th:512 * (th + 1)],
                                start=(c4 == 0), stop=(c4 == 3))
                        with tc.high_priority():
                            nc.vector.tensor_scalar_add(
                                qkt[:, 512 * th:512 * (th + 1)], mm,
                                dq_t[:, fi:fi + 1])
                    qk[fi] = qkt

                # ---- v (token-major with interleaved ones column)
                vaug = []
                for g in range(NG):
                    mm = ps.tile([128, 512], F32, tag="mm", bufs=2,
                                 name=f"vp_{img}_{g}")
                    for c4 in range(4):
                        nc.tensor.matmul(
                            mm, zc[c4][:, 128 * g:128 * (g + 1)],
                            wqkv[c4][:, 2 * C:3 * C],
                            start=(c4 == 0), stop=(c4 == 3))
                    va = sb.tile([128, 33 * NH], BF16, tag="vaug", bufs=8,
                                 name=f"va_{img}_{g}")
                    nc.vector.memset(_ap(va, 32, [[33, NH]]), 1.0)
                    nc.vector.tensor_tensor(
                        out=_ap(va, 0, [[33, NH], [1, 32]]),
                        in0=_ap(mm, 0, [[32, NH], [1, 32]]),
                        in1=_ap(dvb, 0, [[32, NH], [1, 32]]),
                        op=ALU.add)
                    vaug.append(va)
                return qk, vaug

            # ---------------- per-image pipeline ----------------
            _xc0, _zc0 = load_ln1(0)
            _qk0, _va0 = qkv_v(_zc0, 0)
            nxt = (_xc0, _qk0, _va0)
            for img in range(BI):
                xc, qk, vaug = nxt

                # ---- attention, per head-quarter
                atc = [sb.tile([128, 512], BF16, tag="atc", bufs=8,
                               name=f"atc_{img}_{g}") for g in range(NG)]
                for qt in range(4):
                    qh = sb.tile([32, 4 * 1024], BF16, tag="qh", bufs=2,
                                 name=f"qh_{img}_{qt}")
                    kh = sb.tile([32, 4 * 1024], BF16, tag="kh", bufs=1,
                                 name=f"kh_{img}_{qt}")
                    for b4 in range(4):
                        nc.sync.dma_start(
                            out=qh[0:32, 1024 * b4:1024 * (b4 + 1)],
                            in_=qk[qt][32 * b4:32 * (b4 + 1), :])
                        nc.sync.dma_start(
                            out=kh[0:32, 1024 * b4:1024 * (b4 + 1)],
                            in_=qk[4 + qt][32 * b4:32 * (b4 + 1), :])
                    for g in range(NG):
                        stp = ps.tile([128, 512], F32, tag="st", bufs=2,
                                      name=f"stp_{img}_{qt}_{g}")
                        for b4 in range(4):
                            sl = slice(1024 * b4 + 128 * g,
                                       1024 * b4 + 128 * (g + 1))
                            nc.tensor.matmul(
                                stp[:, 128 * b4:128 * (b4 + 1)],
                                kh[0:32, sl], qh[0:32, sl],
                                start=True, stop=True)
                        pt = sb.tile([128, 512], BF16, tag="pt", bufs=3,
                                     name=f"pt_{img}_{qt}_{g}")
                        nc.scalar.activation(pt, stp, AF.Exp)
                        nc.vector.tensor_mul(
                            pt, pt, mask_t[:, 512 * qt:512 * (qt + 1)])
                        av = ps.tile([128, 132], F32, tag="mm", bufs=2,
                                     name=f"av_{img}_{qt}_{g}")
                        for b4 in range(4):
                            h = 4 * qt + b4
                            nc.tensor.matmul(
                                av[:, 33 * b4:33 * (b4 + 1)],
                                pt[:, 128 * b4:128 * (b4 + 1)],
                                vaug[g][:, 33 * h:33 * (h + 1)],
                                start=True, stop=True)
                        rec = sb.tile([128, 4], F32, tag="rec", bufs=2,
                                      name=f"rec_{img}_{qt}_{g}")
                        nc.vector.reciprocal(rec, _ap(av, 32, [[33, 4]]))
                        nc.vector.tensor_tensor(
                            out=_ap(atc[g], 128 * qt, [[32, 4], [1, 32]]),
                            in0=_ap(av, 0, [[33, 4], [1, 32]]),
                            in1=_ap(rec, 0, [[1, 4], [0, 32]]),
                            op=ALU.mult)

                # ---- transpose attention output to channel-major
                actn = []
                for fp in range(4):
                    at = sb.tile([128, 1024], BF16, tag="actn", bufs=4,
                                 name=f"actn_{img}_{fp}")
                    for Q in range(2):
                        tp = ps.tile([128, 512], BF16, tag="av", bufs=2,
                                     name=f"tp_{img}_{fp}_{Q}")
                        for gq in range(4):
                            g = 4 * Q + gq
                            nc.tensor.transpose(
                                tp[:, 128 * gq:128 * (gq + 1)],
                                atc[g][:, 128 * fp:128 * (fp + 1)], idm)
                        nc.scalar.copy(at[:, 512 * Q:512 * (Q + 1)], tp)
                    actn.append(at)

                # ---- proj + residual (in-place xh into xc, window->raster)
                for fo in range(4):
                    for th in range(2):
                        mm = ps.tile([128, 512], F32, tag="mm", bufs=2,
                                     name=f"pj_{img}_{fo}_{th}")
                        for c4 in range(4):
                            nc.tensor.matmul(
                                mm, wproj[c4][:, 128 * fo:128 * (fo + 1)],
                                actn[c4][:, 512 * th:512 * (th + 1)],
                                start=(c4 == 0), stop=(c4 == 3))
                        xap = xc[fo][:, 512 * th:512 * (th + 1)]
                        with tc.high_priority():
                            nc.vector.scalar_tensor_tensor(
                                xap, mm, pb_t[:, fo:fo + 1], xap,
                                ALU.add, ALU.add)

                # prefetch next image's LN1 + qkv/v under this image's tail
                if img + 1 < BI:
                    _xcn, _zcn = load_ln1(img + 1)
                    _qkn, _van = qkv_v(_zcn, img + 1)
                    nxt = (_xcn, _qkn, _van)

                with tc.high_priority():
                    r2_bc, mur2_bc = layernorm(xc, f"l2_{img}")
                z2c = z_pass(xc, r2_bc, mur2_bc, f"l2_{img}")

                # ---- MLP
                for th in range(2):
                    gel = []
                    for f16 in range(16):
                        mm = ps.tile([128, 512], F32, tag="mm", bufs=2,
                                     name=f"m1_{img}_{th}_{f16}")
                        for c4 in range(4):
                            nc.tensor.matmul(
                                mm, wm1[c4][:, 128 * f16:128 * (f16 + 1)],
                                z2c[c4][:, 512 * th:512 * (th + 1)],
                                start=(c4 == 0), stop=(c4 == 3))
                        gt = sb.tile([128, 512], BF16, tag="gelu", bufs=16,
                                     name=f"g_{img}_{th}_{f16}")
                        nc.scalar.activation(gt, mm, AF.Gelu,
                                             bias=d1_t[:, f16:f16 + 1])
                        gel.append(gt)
                    for fo in range(4):
                        mm2 = ps.tile([128, 512], F32, tag="av", bufs=2,
                                      name=f"m2_{img}_{th}_{fo}")
                        for c16 in range(16):
                            nc.tensor.matmul(
                                mm2, wm2[c16][:, 128 * fo:128 * (fo + 1)],
                                gel[c16], start=(c16 == 0), stop=(c16 == 15))
                        xap = xc[fo][:, 512 * th:512 * (th + 1)]
                        nc.vector.scalar_tensor_tensor(
                            xap, mm2, b2_t[:, fo:fo + 1], xap,
                            ALU.add, ALU.add)

                # ---- store (permute window->raster, then contiguous DMA)
                for c4 in range(4):
                    xo = sb.tile([128, 1024], F32, tag="xraw", bufs=2,
                                 name=f"xo_{img}_{c4}")
                    nc.vector.tensor_copy(_ap(xo, 0, RAS8),
                                          _ap(xc[c4], 0, WIN8).bitcast(F32))
                    nc.sync.dma_start(
                        out=out_d[img, 128 * c4:128 * (c4 + 1), :, :]
                        .rearrange("c h w -> c (h w)"),
                        in_=xo)

    nc.compile()
    return nc


def _prep_weights(inputs):
    """Host-side weight preprocessing (numpy, ~ms)."""
    g1 = np.asarray(inputs["norm1_w"], np.float32)
    b1 = np.asarray(inputs["norm1_b"], np.float32)
    g2 = np.asarray(inputs["norm2_w"], np.float32)
    b2n = np.asarray(inputs["norm2_b"], np.float32)
    wqkv = np.array(inputs["qkv_w"], np.float32)              # [3C, C]
    bqkv = np.array(inputs["qkv_b"], np.float32)
    scale = HD ** -0.5
    wqkv[:C] *= scale
    bqkv = bqkv.copy()
    bqkv[:C] *= scale
    dqkv = wqkv @ b1 + bqkv                                   # [3C]
    wqkvT = (wqkv * g1[None, :]).T                            # [C, 3C]

    wproj = np.asarray(inputs["proj_w"], np.float32)          # [C, C]
    pb = np.asarray(inputs["proj_b"], np.float32)
    wm1 = np.asarray(inputs["mlp_w1"], np.float32)            # [MLP_H, C]
    d1 = wm1 @ b2n + np.asarray(inputs["mlp_b1"], np.float32)
    wm1T = (wm1 * g2[None, :]).T                              # [C, MLP_H]
    wm2 = np.asarray(inputs["mlp_w2"], np.float32)            # [C, MLP_H]
    b2o = np.asarray(inputs["mlp_b2"], np.float32)

    rpb = np.asarray(inputs["rpb_table"], np.float32)         # [(2ws-1)^2, NH]
    rel = _relative_position_index(WS)                        # [N, N] (n, m)
    bias = rpb[rel.reshape(-1)].reshape(N, N, NH)             # [n, m, h]
    eb = np.exp(bias)
    mask = np.zeros((128, NH, 128), np.float32)
    for wdx in range(8):
        # tile entry [k, h, q]: k = 16w + m, q = 16w + n -> eb[n, m, h]
        mask[16 * wdx:16 * (wdx + 1), :, 16 * wdx:16 * (wdx + 1)] = \
            eb.transpose(1, 2, 0)
    mask2d = np.ascontiguousarray(
        mask.reshape(128, NH * 128))

    return {
        "wqkv": np.ascontiguousarray(wqkvT).astype(BF),
        "dq": np.ascontiguousarray(
            dqkv[:2 * C].reshape(8, 128).T).astype(np.float32),
        "dvrow": dqkv[2 * C:].reshape(1, C).astype(np.float32),
        "wproj": np.ascontiguousarray(wproj.T).astype(BF),
        "pb": np.ascontiguousarray(pb.reshape(4, 128).T).astype(np.float32),
        "wm1": np.ascontiguousarray(wm1T).astype(BF),
        "d1": np.ascontiguousarray(d1.reshape(16, 128).T).astype(np.float32),
        "wm2": np.ascontiguousarray(wm2.T).astype(BF),
        "b2": np.ascontiguousarray(b2o.reshape(4, 128).T).astype(np.float32),
        "mask": mask2d.astype(BF),
        "idm": np.eye(128, dtype=BF),
        "onesc": np.ones((128, 1), np.float32),
    }


def get_program():
    if "nc" not in _cache:
        _cache["nc"] = _build_program()
    return _cache["nc"]


def make_in_maps(inputs):
    wmaps = _prep_weights(inputs)
    x_full = np.asarray(inputs["x"], np.float32)
    in_maps = []
    for core in range(NCORES):
        m = dict(wmaps)
        m["x"] = np.ascontiguousarray(x_full[BI * core:BI * (core + 1)])
        in_maps.append(m)
    return in_maps


def kernel(**inputs):
    nc = get_program()
    in_maps = make_in_maps(inputs)
    res = run_bass_kernel_spmd(nc, in_maps, list(range(NCORES)))
    out = np.concatenate([res.results[c]["out"] for c in range(NCORES)],
                         axis=0)
    return out
